# revision 1
# baseline (speedup 1.0000x reference)
"""Trainium2 Bass kernel for nn_CBAMSpaceMask (CBAM spatial mask over T timestep blocks).

Math per timestep block t (3 channels):
  mx_c = maxpool3x3(x_c)          (stride 1, -inf pad == replicate pad)
  av_c = avgpool3x3(x_c)/9        (zero pad, count_include_pad)
  y_t  = sum_c wM_c * mx_c + wA_c * av_c + b   (3x3 conv, zero pad)
  out[3t+c] = sigmoid(leakyrelu(y_t))          (broadcast over c)

Design (per core = 1 batch element, pure data parallel over batch):
  - host-side re-layout: input -> [row+2pad, plane, w] bf16, so every tile
    load is a contiguous-3KB-per-partition HBM read and the row-shifted
    U/D tiles for the vertical max are just offset slices (no SBUF->SBUF
    shifts, no load-ordering hazards; pad rows replicate image edges,
    whose conv coefficients are zero)
  - groups of 6 planes (= 2 timesteps, one matmul pair); both row-subs
    (y rows 0:124 and 124:248) share [128, 2, 6, W] tiles
  - pools: vertical 3-row max (2 DVE ops over U/X/D), horizontal 3-tap
    max and box sum (2 DVE ops each) into 258-col zero-padded mx/bh;
    vertical box sum of the avg path folded into the conv operator (op@Bv)
  - conv: banded-Toeplitz matmuls on PE; timestep-paired rhs (planes
    {c, c+3} via stride-3 slice) -> all matmuls full-width N=512, padded
    windows supply the conv zero padding; psum [124, 2, 256]
  - last-8-rows chunk: rows 246..255 packed per quadrant at partition
    bases 0/32/64/96; kw taps folded into K=30 stacked matmuls using R/L
    column-shifted copies; deferred 3 iterations off the critical path
  - epilogue: ACT Prelu(psum+bias, alpha=.01) (same ACT table as sigmoid
    -> no table reloads) -> ACT Sigmoid tripled into all 3 channel slots
    -> ONE bf16 output DMA per epilogue (host upcasts to f32)
  - schedule: loads 3 groups ahead (all wait-free gpsimd software-DGE
    triggers, packets spread over all 16 DMA engines), pools 2 ahead of
    convs, output triggers age 2 checkpoints so no gpsimd trigger ever
    blocks on an incomplete producer
"""
import sys

sys.path.insert(0, "/opt/trn_rl_repo")

import numpy as np
import ml_dtypes
from contextlib import ExitStack

import concourse.bass as bass
import concourse.tile as tile
from concourse import bacc, mybir
from concourse.bass_utils import run_bass_kernel_spmd

F32 = mybir.dt.float32
BF16 = mybir.dt.bfloat16

B, CTOT, H, W = 8, 48, 256, 256
T = 16
N_CORES = 8
NGRP = 8            # groups of 6 planes = 2 timesteps
GP = 6              # planes per group
# main chunk geometry: y rows [m0,m1) from x rows [r0,r1)
SUBS = [(0, 124, 0, 128), (124, 248, 122, 250)]
C2 = (248, 256, 246, 256)   # last-8-rows chunk
NMAIN = 2 * 3 * 3 * 2       # path, c, kw, sub
NC2 = 2 * 3                 # path, c (kw folded into K=30)
NMAT = NMAIN + NC2

_cache = {}


def _build_stack(conv_w):
    """lhsT stack [128, NMAT, 128] bf16.

    mats 0..35: main-sub ops, idx = ((path*3 + c)*3 + kw)*2 + sub,
      lhsT = op[m0:m1, r0:r1].T  ([K=128, M=124])
    mats 36..41: chunk-2 stacked ops, idx = 36 + path*3 + c,
      [K=30, M=8]: K blocks of 10 rows for kw = 1 (center), 0, 2,
      replicated at partition bases 0/32/64/96.
    """
    w = conv_w[0].astype(np.float64)  # [6, 3, 3]
    Bv = np.zeros((H, H))
    for i in (-1, 0, 1):
        Bv += np.eye(H, k=i)
    stack = np.zeros((128, NMAT, 128), dtype=np.float64)

    def band_op(path, c, kw):
        op = np.zeros((H, H))
        k2d = w[2 * c] if path == 0 else w[2 * c + 1]
        for kh in range(3):
            op += k2d[kh, kw] * np.eye(H, k=kh - 1)
        if path == 1:
            op = (op @ Bv) / 9.0
        return op

    for path in range(2):
        for c in range(3):
            for kw in range(3):
                op = band_op(path, c, kw)
                for sub, (m0, m1, r0, r1) in enumerate(SUBS):
                    mat = ((path * 3 + c) * 3 + kw) * 2 + sub
                    lhsT = op[m0:m1, r0:r1].T  # [K, M]
                    K, M = lhsT.shape
                    stack[:K, mat, :M] = lhsT
            # chunk 2: kw-stacked [30, 8]
            mat = NMAIN + path * 3 + c
            m0, m1, r0, r1 = C2
            for kwi, kw in enumerate((1, 0, 2)):
                lhsT = band_op(path, c, kw)[m0:m1, r0:r1].T  # [10, 8]
                for base in (0, 32, 64, 96):
                    stack[base + 10 * kwi:base + 10 * kwi + 10, mat, :8] = lhsT
    return stack.astype(ml_dtypes.bfloat16)


def _mat_main(path, c, kw, sub):
    return ((path * 3 + c) * 3 + kw) * 2 + sub


def _mat_c2(path, c):
    return NMAIN + path * 3 + c


def _prep_x(xi):
    """Host-side re-layout: [plane, row, w] f32 -> [row+pad, plane, w] bf16.

    Row r of the padded tensor holds image row r-1; rows 0 and 257 replicate
    the image edge rows (the maxpool clamp; conv coefficients there are 0).
    With rows outermost, every on-chip tile load is a contiguous-3KB-per-
    partition HBM read, and the row-shifted U/D tiles are just offset slices
    (no SBUF->SBUF shift copies, no load-ordering hazards).
    """
    xp = np.empty((H + 2, CTOT, W), dtype=ml_dtypes.bfloat16)
    xp[1:H + 1] = xi.transpose(1, 0, 2)
    xp[0] = xp[1]
    xp[H + 1] = xp[H]
    return xp


def _build_program():
    nc = bacc.Bacc("TRN2", target_bir_lowering=False, debug=False, enable_asserts=False)
    x_ap = nc.dram_tensor("x", [H + 2, CTOT, W], BF16, kind="ExternalInput").ap()
    cst_ap = nc.dram_tensor("cst", [128, NMAT, 128], BF16, kind="ExternalInput").ap()
    bias_ap = nc.dram_tensor("bias", [128, 1], F32, kind="ExternalInput").ap()
    # bf16 output: halves the output DMA volume through the software queue;
    # sigmoid outputs lie in (0,1) so bf16 quantization error (~0.4% rel) is
    # far inside the accuracy budget. Host upcasts to f32.
    out_ap = nc.dram_tensor("out", [CTOT, H, W], BF16, kind="ExternalOutput").ap()

    MAXOP = mybir.AluOpType.max
    ADDOP = mybir.AluOpType.add

    with tile.TileContext(nc) as tc, ExitStack() as ctx:
        const_pool = ctx.enter_context(tc.tile_pool(name="const", bufs=1))
        psum_pool = ctx.enter_context(tc.tile_pool(name="psum", bufs=6, space="PSUM"))
        epi_pool = ctx.enter_context(tc.tile_pool(name="epi", bufs=4))
        sg_pool = ctx.enter_context(tc.tile_pool(name="sg", bufs=10))
        t2_pool = ctx.enter_context(tc.tile_pool(name="t2", bufs=1))
        x_pool = ctx.enter_context(tc.tile_pool(name="xload", bufs=3))
        ud_pool = ctx.enter_context(tc.tile_pool(name="ud", bufs=3))
        mxbh_pool = ctx.enter_context(tc.tile_pool(name="mxbh", bufs=3))

        cst = const_pool.tile([128, NMAT, 128], BF16, tag="cst")
        nc.sync.dma_start(out=cst[:], in_=cst_ap)
        bias = const_pool.tile([128, 1], F32, tag="bias")
        nc.sync.dma_start(out=bias[:], in_=bias_ap)

        # ---- t2 tiles: rows 246..255 of quadrant q (planes 12q..12q+11) at
        # partitions 32q..32q+9. MX/BH are padded to 258 cols (data at cols
        # 1..256, zero pads) and also hold R/L column-shifted copies at
        # partition offsets +10 / +20 (kw-folded K=30).
        WP = W + 2
        QP = 12  # planes per t2 quadrant (2 groups)
        T2X = t2_pool.tile([128, QP, W], BF16, tag="t2x")
        T2U = t2_pool.tile([128, QP, W], BF16, tag="t2u")
        T2D = t2_pool.tile([128, QP, W], BF16, tag="t2d")
        T2MX = t2_pool.tile([128, QP, WP], BF16, tag="t2mx")
        T2BH = t2_pool.tile([128, QP, WP], BF16, tag="t2bh")
        # zero-fill so gap partitions / pad columns stay finite zeros
        nc.vector.memzero(T2X[:])
        nc.vector.memzero(T2U[:])
        nc.vector.memzero(T2D[:])
        nc.vector.memzero(T2MX[:])
        nc.vector.memzero(T2BH[:])

        # Both row-subs share one tile: X[:, s] holds x rows r0s..r0s+127 of
        # the group's 6 planes (sub 0: rows 0..127, sub 1: rows 122..249).
        # X, U (rows+1), D (rows-1) are all direct offset slices of the
        # host-padded HBM tensor: independent contiguous loads, no chains.
        xud_tiles = {}

        def load_xud(g):
            """HBM->SBUF loads for group g (issued ahead, all wait-free)."""
            X = x_pool.tile([128, 2, GP, W], BF16, tag="x")
            U = ud_pool.tile([128, 2, GP, W], BF16, tag="u")
            D = ud_pool.tile([128, 2, GP, W], BF16, tag="d")
            p0, p1 = GP * g, GP * g + GP
            for sub, (m0, m1, r0, r1) in enumerate(SUBS):
                nc.gpsimd.dma_start(out=X[:, sub], in_=x_ap[r0 + 1:r1 + 1, p0:p1, :])
                nc.gpsimd.dma_start(out=U[:, sub], in_=x_ap[r0 + 2:r1 + 2, p0:p1, :])
                nc.gpsimd.dma_start(out=D[:, sub], in_=x_ap[r0:r1, p0:p1, :])
            xud_tiles[g] = (X, U, D)

        def pools(g):
            """DVE pools for both subs of g."""
            X, U, D = xud_tiles.pop(g)
            # mx/bh padded: data at cols 1..256, cols 0/257 stay zero
            mx = mxbh_pool.tile([128, 2, GP, WP], BF16, tag="mx")
            bh = mxbh_pool.tile([128, 2, GP, WP], BF16, tag="bh")
            nc.vector.memset(mx[:, :, :, 0:1], 0)
            nc.vector.memset(mx[:, :, :, 257:258], 0)
            nc.vector.memset(bh[:, :, :, 0:1], 0)
            nc.vector.memset(bh[:, :, :, 257:258], 0)
            # bh first: it only needs X, so DVE proceeds while the U/D
            # shift DMAs are still in flight
            nc.vector.tensor_tensor(out=bh[:, :, :, 1:256], in0=X[:, :, :, 0:255],
                                    in1=X[:, :, :, 1:256], op=ADDOP)
            nc.vector.tensor_copy(bh[:, :, :, 256:257], X[:, :, :, 255:256])
            nc.vector.tensor_tensor(out=bh[:, :, :, 2:257], in0=bh[:, :, :, 2:257],
                                    in1=X[:, :, :, 0:255], op=ADDOP)
            # vertical 3-row max (DVE), in place into U
            vx = U
            nc.vector.tensor_tensor(out=vx[:], in0=U[:], in1=D[:], op=MAXOP)
            nc.vector.tensor_tensor(out=vx[:], in0=vx[:], in1=X[:], op=MAXOP)
            # horizontal 3-tap max (DVE) into padded mx
            nc.vector.tensor_tensor(out=mx[:, :, :, 1:256], in0=vx[:, :, :, 0:255],
                                    in1=vx[:, :, :, 1:256], op=MAXOP)
            nc.vector.tensor_copy(mx[:, :, :, 256:257], vx[:, :, :, 255:256])
            nc.vector.tensor_tensor(out=mx[:, :, :, 2:257], in0=mx[:, :, :, 2:257],
                                    in1=vx[:, :, :, 0:255], op=MAXOP)
            return mx, bh

        def load_t2(q):
            b = 32 * q
            m0, m1, r0, r1 = C2
            p0, p1 = QP * q, QP * q + QP
            nc.sync.dma_start(out=T2X[b:b + 10], in_=x_ap[r0 + 1:r1 + 1, p0:p1, :])
            nc.sync.dma_start(out=T2U[b:b + 10], in_=x_ap[r0 + 2:r1 + 2, p0:p1, :])
            nc.sync.dma_start(out=T2D[b:b + 10], in_=x_ap[r0:r1, p0:p1, :])

        def t2_pools():
            """Pools over the whole packed tile (all 4 quadrants at once)."""
            vx = T2U
            nc.vector.tensor_tensor(out=vx[:], in0=T2U[:], in1=T2D[:], op=MAXOP)
            nc.vector.tensor_tensor(out=vx[:], in0=vx[:], in1=T2X[:], op=MAXOP)
            nc.vector.tensor_tensor(out=T2MX[:, :, 1:256], in0=vx[:, :, 0:255],
                                    in1=vx[:, :, 1:256], op=MAXOP)
            nc.vector.tensor_copy(T2MX[:, :, 256:257], vx[:, :, 255:256])
            nc.vector.tensor_tensor(out=T2MX[:, :, 2:257], in0=T2MX[:, :, 2:257],
                                    in1=vx[:, :, 0:255], op=MAXOP)
            nc.vector.tensor_tensor(out=T2BH[:, :, 1:256], in0=T2X[:, :, 0:255],
                                    in1=T2X[:, :, 1:256], op=ADDOP)
            nc.vector.tensor_copy(T2BH[:, :, 256:257], T2X[:, :, 255:256])
            nc.vector.tensor_tensor(out=T2BH[:, :, 2:257], in0=T2BH[:, :, 2:257],
                                    in1=T2X[:, :, 0:255], op=ADDOP)
            # R/L column-shifted copies into partition blocks +10 / +20.
            # All matmul rhs windows read cols 1..256 of their block:
            #   block +10 pairs kw=0 (needs P[w-1]): dest col j <- data col j-1
            #   block +20 pairs kw=2 (needs P[w+1]): dest col j <- data col j+1
            # Pad columns supply the conv zero padding.
            for q in range(4):
                b = 32 * q
                for tl in (T2MX, T2BH):
                    nc.sync.dma_start(out=tl[b + 10:b + 20, :, 2:257],
                                      in_=tl[b:b + 10, :, 1:256])
                    nc.sync.dma_start(out=tl[b + 20:b + 30, :, 1:256],
                                      in_=tl[b:b + 10, :, 2:257])

        KW_ORDER = [(path, c, kw)
                    for c in range(3) for path in range(2) for kw in (1, 0, 2)]
        KW_ORDER.sort(key=lambda pck: 0 if pck[2] == 1 else 1)

        out_ready = []   # sigmoids surely complete: safe to issue triggers
        out_recent = []  # freshly issued sigmoids: age one checkpoint first

        def epilogue_lrelu(ps, M):
            """ACT Lrelu(psum + bias) -> lk. Sigmoids are batched separately
            per iteration so the ACT engine swaps function tables only twice
            per group instead of per epilogue."""
            lk = epi_pool.tile([128, 2, W], F32, tag="epil")
            # Prelu (parametric_relu) lives in the same ACT function table as
            # sigmoid, so alternating them costs no ACT_TABLE_LOADs
            nc.scalar.activation(lk[0:M], ps[0:M],
                                 mybir.ActivationFunctionType.Prelu,
                                 bias=bias[0:M], scale=1.0, alpha=0.01)
            return lk

        def epilogue_sigmoid(lk, M, p0, m0, m1):
            """The sigmoid writes each value to all 3 channel slots of sg so
            ONE output DMA covers both timesteps. Output DMA issue is
            DEFERRED (gpsimd queue is in-order: a trigger waiting on its
            sigmoid would stall later load triggers)."""
            sg = sg_pool.tile([128, 2, 3, W], BF16, tag="epis")
            nc.scalar.activation(sg[0:M],
                                 lk[0:M].unsqueeze(2).to_broadcast([M, 2, 3, W]),
                                 mybir.ActivationFunctionType.Sigmoid)
            out_recent.append((sg, M, p0, m0, m1))

        def flush_outputs(final=False):
            for sg, M, p0, m0, m1 in out_ready:
                dst = out_ap[p0:p0 + 6, m0:m1, :].transpose([1, 0, 2])
                nc.gpsimd.dma_start(out=dst, in_=sg[0:M])
            out_ready.clear()
            out_ready.extend(out_recent)
            out_recent.clear()
            if final and out_ready:
                flush_outputs()

        def conv_sub(g, sub, mx, bh):
            """Main-chunk accumulation for group g's timestep pair.

            All 18 matmuls are full-width N=512: the kw tap s reads the
            padded rhs window cols (1+s)..(256+s), whose zero pads supply
            the conv zero padding.
            """
            m0, m1, r0, r1 = SUBS[sub]
            M, K = m1 - m0, r1 - r0
            ps = psum_pool.tile([128, 2, W], F32, tag="ps")
            n = len(KW_ORDER)
            for i, (path, c, kw) in enumerate(KW_ORDER):
                s = kw - 1
                mat = _mat_main(path, c, kw, sub)
                srcs = mx if path == 0 else bh
                rhs = srcs[0:K, sub, c:c + 4:3, 1 + s:257 + s]
                nc.tensor.matmul(ps[0:M], cst[0:K, mat, 0:M], rhs,
                                 start=(i == 0), stop=(i == n - 1))
            return epilogue_lrelu(ps, M), M, GP * g, m0, m1

        def conv_c2(g):
            """Last-8-rows accumulation (kw-folded, K=30) for group g."""
            m0, m1, r0, r1 = C2
            M = m1 - m0
            b = 32 * (g // 2)
            pb = 6 * (g % 2)
            ps = psum_pool.tile([128, 2, W], F32, tag="ps")
            idx = 0
            for path in range(2):
                for c in range(3):
                    mat = _mat_c2(path, c)
                    src = T2MX if path == 0 else T2BH
                    rhs = src[b:b + 30, pb + c:pb + c + 4:3, 1:257]
                    nc.tensor.matmul(ps[0:M], cst[b:b + 30, mat, 0:M], rhs,
                                     start=(idx == 0), stop=(idx == NC2 - 1),
                                     tile_position=(b, 0))
                    idx += 1
            return epilogue_lrelu(ps, M), M, GP * g, m0, m1

        # ---- schedule: X loads run 3 groups ahead; pools for g+1 are issued
        # at the top of iteration g, ahead of the epilogue STTs in the DVE
        # queue, so they execute while PE runs group g's convs. Output
        # triggers age through two checkpoints before issue so no gpsimd
        # trigger ever waits on an incomplete producer.
        # conv_c2(g) is deferred two iterations (it only needs the t2 pools
        # and a psum bank), so t2 pool work stays off the early critical path
        for g in range(3):
            load_xud(g)
        pools_of = {0: pools(0), 1: pools(1)}
        for q in range(4):
            load_t2(q)
        t2_pools()
        for g in range(NGRP):
            if g + 3 < NGRP:
                load_xud(g + 3)
            if g + 2 < NGRP:
                pools_of[g + 2] = pools(g + 2)
            mx, bh = pools_of.pop(g)
            epilogue_sigmoid(*conv_sub(g, 0, mx, bh))
            epilogue_sigmoid(*conv_sub(g, 1, mx, bh))
            flush_outputs()
            if g >= 3:
                epilogue_sigmoid(*conv_c2(g - 3))
        for g in (NGRP - 3, NGRP - 2, NGRP - 1):
            epilogue_sigmoid(*conv_c2(g))
        flush_outputs(final=True)

    nc.compile()
    return nc


def kernel(input_tensor, conv_w, conv_b):
    input_tensor = np.ascontiguousarray(np.asarray(input_tensor, dtype=np.float32))
    conv_w = np.asarray(conv_w, dtype=np.float32)
    conv_b = np.asarray(conv_b, dtype=np.float32)

    if "nc" not in _cache:
        _cache["nc"] = _build_program()
    nc = _cache["nc"]

    stack = _build_stack(conv_w)
    bias_vec = np.full((128, 1), conv_b[0], dtype=np.float32)
    in_maps = [
        {"x": _prep_x(input_tensor[i]), "cst": stack, "bias": bias_vec}
        for i in range(N_CORES)
    ]
    res = run_bass_kernel_spmd(nc, in_maps, list(range(N_CORES)))
    out = np.stack([res.results[i]["out"] for i in range(N_CORES)], axis=0)
    return out.astype(np.float32)


if __name__ == "__main__":
    rng = np.random.default_rng(0)
    x = rng.standard_normal((B, CTOT, H, W), dtype=np.float32)
    cw = rng.uniform(-0.1, 0.1, (1, 6, 3, 3)).astype(np.float32)
    cb = np.array([0.01], dtype=np.float32)
    o = kernel(x, cw, cb)
    print(o.shape, o.dtype)



# revision 7
# speedup vs baseline: 1.0961x; 1.0961x over previous
"""Trainium2 Bass kernel for nn_CBAMSpaceMask (CBAM spatial mask over T timestep blocks).

Math per timestep block t (3 channels):
  mx_c = maxpool3x3(x_c)          (stride 1, -inf pad == replicate pad)
  av_c = avgpool3x3(x_c)/9        (zero pad, count_include_pad)
  y_t  = sum_c wM_c * mx_c + wA_c * av_c + b   (3x3 conv, zero pad)
  out[3t+c] = sigmoid(leakyrelu(y_t))          (broadcast over c)

Design (per core = 1 batch element, pure data parallel over batch):
  - host-side re-layout: input -> [row+2pad, plane, w] bf16; pad rows replicate
    image edges (maxpool -inf-pad clamp; conv coefficients there are zero)
  - ONE overlapping-AP DMA per group loads a [128, 2sub, 3shift, 6, W] tile:
    shift j supplies row p+j-1 on partition p (HBM APs are flat, so the row
    stride can repeat across the shift dim); this replaces 3 separate X/U/D
    loads per sub (1 gpsimd software-DGE trigger per group instead of 6)
  - groups of 6 planes (= 2 timesteps, one matmul pair); both row-subs
    (y rows 0:124 and 124:248) share the tile
  - pools: vertical 3-row max (2 DVE ops over the shift slices), horizontal
    3-tap max and box sum (2 DVE ops each) into 258-col zero-padded mx/bh;
    vertical box sum of the avg path folded into the conv operator (op@Bv);
    pad-column zeroing via ONE strided-AP memset per tile (cols 0 and 257)
  - conv: banded-Toeplitz matmuls on PE; timestep-paired rhs (planes
    {c, c+3} via stride-3 slice) -> all matmuls full-width N=512, padded
    windows supply the conv zero padding; psum [124, 2, 256]; max-path
    matmuls issued first so the bh pools may lag the mx pools
  - last-8-rows chunk: rows 246..255 packed per quadrant at partition
    bases 0/32/64/96; kw taps folded into K=30 stacked matmuls using R/L
    column-shifted copies; deferred 3 iterations off the critical path
  - epilogue: ACT Prelu(psum+bias, alpha=.01) IN PLACE on psum (same ACT
    table as sigmoid -> no table reloads) -> ACT Sigmoid -> bf16 sg tile.
    Output is ONE channel per timestep ([T, H, W]); the host broadcasts to
    the 3 channels (reference broadcasts before the elementwise sigmoid, so
    results are identical) -> output DMA volume and sigmoid work cut 3x
  - c2 sigmoids write a persistent [8, 16, W] tile; ONE final DMA stores
    rows 248..255 for all timesteps
  - schedule: loads 3 groups ahead (wait-free gpsimd software-DGE triggers),
    pools 2 ahead of convs, output triggers age 2 checkpoints so no gpsimd
    trigger ever blocks on an incomplete producer; group 0 is loaded and
    pooled per-sub with dedicated tiles so PE starts ~8us into the run
"""
import sys

sys.path.insert(0, "/opt/trn_rl_repo")

import numpy as np
import ml_dtypes
from contextlib import ExitStack

import concourse.bass as bass
import concourse.tile as tile
from concourse import bacc, mybir
from concourse.bass_utils import run_bass_kernel_spmd

F32 = mybir.dt.float32
BF16 = mybir.dt.bfloat16

B, CTOT, H, W = 8, 48, 256, 256
T = 16
N_CORES = 8
NGRP = 8            # groups of 6 planes = 2 timesteps
GP = 6              # planes per group
# main chunk geometry: y rows [m0,m1) from x rows [r0,r1)
SUBS = [(0, 124, 0, 128), (124, 248, 122, 250)]
C2 = (248, 256, 246, 256)   # last-8-rows chunk
NMAIN = 2 * 3 * 3 * 2       # path, c, kw, sub
NC2 = 2 * 3                 # path, c (kw folded into K=30)
NMAT = NMAIN + NC2

_cache = {}


def _build_stack(conv_w):
    """lhsT stack [128, NMAT, 128] bf16.

    mats 0..35: main-sub ops, idx = ((path*3 + c)*3 + kw)*2 + sub,
      lhsT = op[m0:m1, r0:r1].T  ([K=128, M=124])
    mats 36..41: chunk-2 stacked ops, idx = 36 + path*3 + c,
      [K=30, M=8]: K blocks of 10 rows for kw = 1 (center), 0, 2,
      replicated at partition bases 0/32/64/96.
    """
    w = conv_w[0].astype(np.float64)  # [6, 3, 3]
    Bv = np.zeros((H, H))
    for i in (-1, 0, 1):
        Bv += np.eye(H, k=i)
    stack = np.zeros((128, NMAT, 128), dtype=np.float64)

    def band_op(path, c, kw):
        op = np.zeros((H, H))
        k2d = w[2 * c] if path == 0 else w[2 * c + 1]
        for kh in range(3):
            op += k2d[kh, kw] * np.eye(H, k=kh - 1)
        if path == 1:
            op = (op @ Bv) / 9.0
        return op

    for path in range(2):
        for c in range(3):
            for kw in range(3):
                op = band_op(path, c, kw)
                for sub, (m0, m1, r0, r1) in enumerate(SUBS):
                    mat = ((path * 3 + c) * 3 + kw) * 2 + sub
                    lhsT = op[m0:m1, r0:r1].T  # [K, M]
                    K, M = lhsT.shape
                    stack[:K, mat, :M] = lhsT
            # chunk 2: kw-stacked [30, 8]
            mat = NMAIN + path * 3 + c
            m0, m1, r0, r1 = C2
            for kwi, kw in enumerate((1, 0, 2)):
                lhsT = band_op(path, c, kw)[m0:m1, r0:r1].T  # [10, 8]
                for base in (0, 32, 64, 96):
                    stack[base + 10 * kwi:base + 10 * kwi + 10, mat, :8] = lhsT
    return stack.astype(ml_dtypes.bfloat16)


def _mat_main(path, c, kw, sub):
    return ((path * 3 + c) * 3 + kw) * 2 + sub


def _mat_c2(path, c):
    return NMAIN + path * 3 + c


def _prep_x(xi):
    """Host-side re-layout: [plane, row, w] f32 -> [row+pad, plane, w] bf16.

    Row r of the padded tensor holds image row r-1; rows 0 and 257 replicate
    the image edge rows (the maxpool clamp; conv coefficients there are 0).
    With rows outermost, every on-chip tile load is a contiguous HBM read per
    partition, and the row-shifted slices are offset views of the same rows.
    """
    xp = np.empty((H + 2, CTOT, W), dtype=ml_dtypes.bfloat16)
    xp[1:H + 1] = xi.transpose(1, 0, 2)
    xp[0] = xp[1]
    xp[H + 1] = xp[H]
    return xp


def _build_program():
    nc = bacc.Bacc("TRN2", target_bir_lowering=False, debug=False, enable_asserts=False)
    x_ap = nc.dram_tensor("x", [H + 2, CTOT, W], BF16, kind="ExternalInput").ap()
    cst_ap = nc.dram_tensor("cst", [128, NMAT, 128], BF16, kind="ExternalInput").ap()
    bias_ap = nc.dram_tensor("bias", [128, 1], F32, kind="ExternalInput").ap()
    # bf16, one channel per timestep: sigmoid outputs lie in (0,1) so bf16
    # quantization (~0.4% rel) is far inside the accuracy budget; the host
    # upcasts to f32 and broadcasts each timestep mask to its 3 channels.
    out_ap = nc.dram_tensor("out", [T, H, W], BF16, kind="ExternalOutput").ap()

    MAXOP = mybir.AluOpType.max
    ADDOP = mybir.AluOpType.add
    RWST = CTOT * W          # HBM row stride (elements)

    with tile.TileContext(nc) as tc, ExitStack() as ctx:
        const_pool = ctx.enter_context(tc.tile_pool(name="const", bufs=1))
        psum_pool = ctx.enter_context(tc.tile_pool(name="psum", bufs=6, space="PSUM"))
        sg_pool = ctx.enter_context(tc.tile_pool(name="sg", bufs=10))
        t2_pool = ctx.enter_context(tc.tile_pool(name="t2", bufs=1))
        x_pool = ctx.enter_context(tc.tile_pool(name="xload", bufs=3))
        mxbh_pool = ctx.enter_context(tc.tile_pool(name="mxbh", bufs=3))
        g0_pool = ctx.enter_context(tc.tile_pool(name="g0", bufs=1))

        cst = const_pool.tile([128, NMAT, 128], BF16, tag="cst")
        nc.sync.dma_start(out=cst[:], in_=cst_ap)
        bias = const_pool.tile([128, 1], F32, tag="bias")
        nc.sync.dma_start(out=bias[:], in_=bias_ap)

        # ---- t2 tiles: rows 246..255 of quadrant q (planes 12q..12q+11) at
        # partitions 32q..32q+9. MX/BH are padded to 258 cols (data at cols
        # 1..256, zero pads) and also hold R/L column-shifted copies at
        # partition offsets +10 / +20 (kw-folded K=30).
        WP = W + 2
        QP = 12  # planes per t2 quadrant (2 groups)
        T2X = t2_pool.tile([128, QP, W], BF16, tag="t2x")
        T2U = t2_pool.tile([128, QP, W], BF16, tag="t2u")
        T2D = t2_pool.tile([128, QP, W], BF16, tag="t2d")
        T2MX = t2_pool.tile([128, QP, WP], BF16, tag="t2mx")
        T2BH = t2_pool.tile([128, QP, WP], BF16, tag="t2bh")
        # c2 sigmoid accumulator: rows 248..255 x all 16 timesteps; ONE
        # final DMA stores it
        C2OUT = t2_pool.tile([128, T, W], BF16, tag="c2out")
        # (no full-tile zeroing: garbage in gap partitions only flows into
        # regions later overwritten by the shift DMAs or never read; the
        # pad columns that ARE read get strided memsets in t2_pools)

        # ---- fused input load: tile [128, 2 sub, 3 shift, GP, W]; shift j
        # holds padded rows (r0_sub + p + j) so the vertical 3-max is three
        # aligned slices of ONE tile. The HBM source AP repeats the row
        # stride across the shift dim (flat DRAM addressing): one software
        # DGE trigger per group.
        xud_tiles = {}

        def _src_ap(g, subs=(0, 1)):
            dims = [[RWST, 128]]
            if len(subs) == 2:
                dims.append([SUBS[1][2] * RWST, 2])
            dims += [[RWST, 3], [W, GP], [1, W]]
            off = GP * g * W + SUBS[subs[0]][2] * RWST
            return bass.AP(x_ap.tensor, off, dims)

        def load_xud(g):
            X = x_pool.tile([128, 2, 3, GP, W], BF16, tag="x")
            for sub in range(2):
                nc.gpsimd.dma_start(out=X[:, sub:sub + 1], in_=_src_ap(g, (sub,)))
            xud_tiles[g] = X

        def pools(g, X=None, mx=None, bh=None, sub=None):
            """DVE pools; when sub is given, operate on that sub slice only
            (used for group 0's fast start with dedicated tiles)."""
            if X is None:
                X = xud_tiles.pop(g)
            if mx is None:
                mx = mxbh_pool.tile([128, 2, GP, WP], BF16, tag="mx")
                bh = mxbh_pool.tile([128, 2, GP, WP], BF16, tag="bh")
            s = slice(None) if sub is None else slice(sub, sub + 1)
            D, XC, U = X[:, s, 0], X[:, s, 1], X[:, s, 2]
            # one strided memset zeroes both pad columns (0 and 257)
            nc.vector.memset(mx[:, s, :, 0:258:257], 0)
            nc.vector.memset(bh[:, s, :, 0:258:257], 0)
            # vertical 3-row max (DVE), in place into the U slice
            vx = U
            nc.vector.tensor_tensor(out=vx, in0=U, in1=D, op=MAXOP)
            nc.vector.tensor_tensor(out=vx, in0=vx, in1=XC, op=MAXOP)
            # horizontal 3-tap max (DVE) into padded mx
            nc.vector.tensor_tensor(out=mx[:, s, :, 1:256], in0=vx[:, :, :, 0:255],
                                    in1=vx[:, :, :, 1:256], op=MAXOP)
            nc.vector.tensor_copy(mx[:, s, :, 256:257], vx[:, :, :, 255:256])
            nc.vector.tensor_tensor(out=mx[:, s, :, 2:257], in0=mx[:, s, :, 2:257],
                                    in1=vx[:, :, :, 0:255], op=MAXOP)
            # horizontal 3-tap box sum (DVE) into padded bh
            nc.vector.tensor_tensor(out=bh[:, s, :, 1:256], in0=XC[:, :, :, 0:255],
                                    in1=XC[:, :, :, 1:256], op=ADDOP)
            nc.vector.tensor_copy(bh[:, s, :, 256:257], XC[:, :, :, 255:256])
            nc.vector.tensor_tensor(out=bh[:, s, :, 2:257], in0=bh[:, s, :, 2:257],
                                    in1=XC[:, :, :, 0:255], op=ADDOP)
            return mx, bh

        def load_t2(q):
            b = 32 * q
            m0, m1, r0, r1 = C2
            p0, p1 = QP * q, QP * q + QP
            nc.sync.dma_start(out=T2X[b:b + 10], in_=x_ap[r0 + 1:r1 + 1, p0:p1, :])
            nc.sync.dma_start(out=T2U[b:b + 10], in_=x_ap[r0 + 2:r1 + 2, p0:p1, :])
            nc.sync.dma_start(out=T2D[b:b + 10], in_=x_ap[r0:r1, p0:p1, :])

        def t2_pools():
            """Pools over the whole packed tile (all 4 quadrants at once)."""
            vx = T2U
            nc.vector.tensor_tensor(out=vx[:], in0=T2U[:], in1=T2D[:], op=MAXOP)
            nc.vector.tensor_tensor(out=vx[:], in0=vx[:], in1=T2X[:], op=MAXOP)
            # zero the pad columns 0/257 (one strided memset per tile); the
            # widened shifts below carry the zero edge into col 1 of the
            # +10 blocks and col 256 of the +20 blocks (conv zero pad)
            nc.vector.memset(T2MX[:, :, 0:258:257], 0)
            nc.vector.memset(T2BH[:, :, 0:258:257], 0)
            nc.vector.tensor_tensor(out=T2MX[:, :, 1:256], in0=vx[:, :, 0:255],
                                    in1=vx[:, :, 1:256], op=MAXOP)
            nc.vector.tensor_copy(T2MX[:, :, 256:257], vx[:, :, 255:256])
            nc.vector.tensor_tensor(out=T2MX[:, :, 2:257], in0=T2MX[:, :, 2:257],
                                    in1=vx[:, :, 0:255], op=MAXOP)
            nc.vector.tensor_tensor(out=T2BH[:, :, 1:256], in0=T2X[:, :, 0:255],
                                    in1=T2X[:, :, 1:256], op=ADDOP)
            nc.vector.tensor_copy(T2BH[:, :, 256:257], T2X[:, :, 255:256])
            nc.vector.tensor_tensor(out=T2BH[:, :, 2:257], in0=T2BH[:, :, 2:257],
                                    in1=T2X[:, :, 0:255], op=ADDOP)
            # R/L column-shifted copies into partition blocks +10 / +20.
            # All matmul rhs windows read cols 1..256 of their block:
            #   block +10 pairs kw=0 (needs P[w-1]): dest col j <- data col j-1
            #   block +20 pairs kw=2 (needs P[w+1]): dest col j <- data col j+1
            for q in range(4):
                b = 32 * q
                for tl in (T2MX, T2BH):
                    nc.sync.dma_start(out=tl[b + 10:b + 20, :, 1:257],
                                      in_=tl[b:b + 10, :, 0:256])
                    nc.sync.dma_start(out=tl[b + 20:b + 30, :, 1:257],
                                      in_=tl[b:b + 10, :, 2:258])

        # max path first: its matmuls depend only on mx, so PE can start
        # while the bh pools are still in the DVE queue
        KW_ORDER = [(path, c, kw)
                    for path in range(2) for c in range(3) for kw in (1, 0, 2)]

        out_ready = []   # sigmoids surely complete: safe to issue triggers
        out_recent = []  # freshly issued sigmoids: age one checkpoint first

        def epilogue_lrelu(ps, M):
            """ACT Prelu(psum + bias) IN PLACE on the psum bank. Prelu
            (parametric_relu) lives in the same ACT function table as
            sigmoid, so alternating them costs no ACT_TABLE_LOADs."""
            nc.scalar.activation(ps[0:M], ps[0:M],
                                 mybir.ActivationFunctionType.Prelu,
                                 bias=bias[0:M], scale=1.0, alpha=0.01)
            return ps

        def epilogue_sigmoid(ps, M, t0, m0, m1):
            """Sigmoid psum -> bf16 sg (one channel per timestep). Output
            DMA issue is DEFERRED (gpsimd queue is in-order: a trigger
            waiting on its sigmoid would stall later load triggers)."""
            sg = sg_pool.tile([128, 2, W], BF16, tag="epis")
            nc.scalar.activation(sg[0:M], ps[0:M],
                                 mybir.ActivationFunctionType.Sigmoid)
            out_recent.append((sg, M, t0, m0, m1))

        def flush_outputs(final=False):
            for sg, M, t0, m0, m1 in out_ready:
                dst = out_ap[t0:t0 + 2, m0:m1, :].transpose([1, 0, 2])
                nc.gpsimd.dma_start(out=dst, in_=sg[0:M])
            out_ready.clear()
            out_ready.extend(out_recent)
            out_recent.clear()
            if final and out_ready:
                flush_outputs()

        def conv_sub(g, sub, mx, bh):
            """Main-chunk accumulation for group g's timestep pair.

            All 18 matmuls are full-width N=512: the kw tap s reads the
            padded rhs window cols (1+s)..(256+s), whose zero pads supply
            the conv zero padding.
            """
            m0, m1, r0, r1 = SUBS[sub]
            M, K = m1 - m0, r1 - r0
            ps = psum_pool.tile([128, 2, W], F32, tag="ps")
            n = len(KW_ORDER)
            for i, (path, c, kw) in enumerate(KW_ORDER):
                s = kw - 1
                mat = _mat_main(path, c, kw, sub)
                srcs = mx if path == 0 else bh
                sb = 0 if mx.shape[1] == 1 else sub
                rhs = srcs[0:K, sb, c:c + 4:3, 1 + s:257 + s]
                nc.tensor.matmul(ps[0:M], cst[0:K, mat, 0:M], rhs,
                                 start=(i == 0), stop=(i == n - 1))
            epilogue_lrelu(ps, M)
            return ps, M, 2 * g, m0, m1

        def conv_c2(g):
            """Last-8-rows accumulation (kw-folded, K=30) for group g; the
            sigmoid lands in the persistent C2OUT tile."""
            m0, m1, r0, r1 = C2
            M = m1 - m0
            b = 32 * (g // 2)
            pb = 6 * (g % 2)
            ps = psum_pool.tile([128, 2, W], F32, tag="ps")
            idx = 0
            for path in range(2):
                for c in range(3):
                    mat = _mat_c2(path, c)
                    src = T2MX if path == 0 else T2BH
                    rhs = src[b:b + 30, pb + c:pb + c + 4:3, 1:257]
                    nc.tensor.matmul(ps[0:M], cst[b:b + 30, mat, 0:M], rhs,
                                     start=(idx == 0), stop=(idx == NC2 - 1),
                                     tile_position=(b, 0))
                    idx += 1
            epilogue_lrelu(ps, M)
            nc.scalar.activation(C2OUT[0:M, 2 * g:2 * g + 2], ps[0:M],
                                 mybir.ActivationFunctionType.Sigmoid)

        # ---- schedule: group 0 is loaded per-sub with dedicated pool tiles
        # so the first matmul only waits on sub 0's load + 6 DVE ops. Later
        # loads run 3 groups ahead; pools for g+2 are issued at the top of
        # iteration g so they execute while PE runs group g's convs. Output
        # triggers age through two checkpoints before issue.
        # conv_c2(g) is deferred three iterations (it only needs the t2 pools
        # and a psum bank), so t2 pool work stays off the early critical path
        g0X = g0_pool.tile([128, 2, 3, GP, W], BF16, tag="g0x")
        g0t = []
        for sub in range(2):
            nc.gpsimd.dma_start(out=g0X[:, sub:sub + 1], in_=_src_ap(0, (sub,)))
            mxs = g0_pool.tile([128, 1, GP, WP], BF16, tag=f"g0mx{sub}")
            bhs = g0_pool.tile([128, 1, GP, WP], BF16, tag=f"g0bh{sub}")
            pools(0, X=g0X[:, sub:sub + 1], mx=mxs, bh=bhs, sub=0)
            g0t.append((mxs, bhs))
        for g in (1, 2):
            load_xud(g)
        pools_of = {1: pools(1), 2: pools(2)}
        for q in range(4):
            load_t2(q)
        # c2 deferral: conv_c2(g) runs 4 iterations later, after the t2
        # pools (issued at g=1, behind pools(3) in the DVE queue) are done
        for g in range(NGRP):
            if g + 3 < NGRP:
                load_xud(g + 3)
            if g + 2 < NGRP and g >= 1:
                pools_of[g + 2] = pools(g + 2)
            if g == 0:
                epilogue_sigmoid(*conv_sub(0, 0, *g0t[0]))
                epilogue_sigmoid(*conv_sub(0, 1, *g0t[1]))
            else:
                mx, bh = pools_of.pop(g)
                epilogue_sigmoid(*conv_sub(g, 0, mx, bh))
                epilogue_sigmoid(*conv_sub(g, 1, mx, bh))
            if g == 1:
                t2_pools()
            flush_outputs()
            if g >= 4:
                conv_c2(g - 4)
        for g in (NGRP - 4, NGRP - 3, NGRP - 2, NGRP - 1):
            conv_c2(g)
        flush_outputs(final=True)
        # one DMA for all last-8 rows: [T, 8, W] <- C2OUT[0:8] transposed
        m0 = C2[0]
        nc.sync.dma_start(out=out_ap[:, m0:m0 + 8, :].transpose([1, 0, 2]),
                          in_=C2OUT[0:8])

    nc.compile()
    return nc


def kernel(input_tensor, conv_w, conv_b):
    input_tensor = np.ascontiguousarray(np.asarray(input_tensor, dtype=np.float32))
    conv_w = np.asarray(conv_w, dtype=np.float32)
    conv_b = np.asarray(conv_b, dtype=np.float32)

    if "nc" not in _cache:
        _cache["nc"] = _build_program()
    nc = _cache["nc"]

    stack = _build_stack(conv_w)
    bias_vec = np.full((128, 1), conv_b[0], dtype=np.float32)
    in_maps = [
        {"x": _prep_x(input_tensor[i]), "cst": stack, "bias": bias_vec}
        for i in range(N_CORES)
    ]
    res = run_bass_kernel_spmd(nc, in_maps, list(range(N_CORES)))
    # [T, H, W] bf16 per core -> broadcast each timestep mask to 3 channels
    out = np.stack([res.results[i]["out"] for i in range(N_CORES)], axis=0)
    out = np.repeat(out.astype(np.float32), 3, axis=1)
    return out


if __name__ == "__main__":
    rng = np.random.default_rng(0)
    x = rng.standard_normal((B, CTOT, H, W), dtype=np.float32)
    cw = rng.uniform(-0.1, 0.1, (1, 6, 3, 3)).astype(np.float32)
    cb = np.array([0.01], dtype=np.float32)
    o = kernel(x, cw, cb)
    print(o.shape, o.dtype)


# revision 19
# speedup vs baseline: 1.0995x; 1.0030x over previous
"""Trainium2 Bass kernel for nn_CBAMSpaceMask (CBAM spatial mask over T timestep blocks).

Math per timestep block t (3 channels):
  mx_c = maxpool3x3(x_c)          (stride 1, -inf pad == replicate pad)
  av_c = avgpool3x3(x_c)/9        (zero pad, count_include_pad)
  y_t  = sum_c wM_c * mx_c + wA_c * av_c + b   (3x3 conv, zero pad)
  out[3t+c] = sigmoid(leakyrelu(y_t))          (broadcast over c)

Design (per core = 1 batch element, pure data parallel over batch):
  - host-side re-layout: input -> [row+2pad, plane, w] bf16; pad rows replicate
    image edges (maxpool -inf-pad clamp; conv coefficients there are zero)
  - ONE overlapping-AP DMA per group loads a [128, 2sub, 3shift, 6, W] tile:
    shift j supplies row p+j-1 on partition p (HBM APs are flat, so the row
    stride can repeat across the shift dim); this replaces 3 separate X/U/D
    loads per sub (1 gpsimd software-DGE trigger per group instead of 6)
  - groups of 6 planes (= 2 timesteps, one matmul pair); both row-subs
    (y rows 0:124 and 124:248) share the tile
  - pools: vertical 3-row max (2 DVE ops over the shift slices), horizontal
    3-tap max and box sum (2 DVE ops each) into 258-col zero-padded mx/bh;
    vertical box sum of the avg path folded into the conv operator (op@Bv);
    pad-column zeroing via ONE strided-AP memset per tile (cols 0 and 257)
  - conv: banded-Toeplitz matmuls on PE; timestep-paired rhs (planes
    {c, c+3} via stride-3 slice) -> all matmuls full-width N=512, padded
    windows supply the conv zero padding; psum [124, 2, 256]; max-path
    matmuls issued first so the bh pools may lag the mx pools
  - last-8-rows chunk: rows 246..255 packed per quadrant at partition
    bases 0/32/64/96; kw taps folded into K=30 stacked matmuls using R/L
    column-shifted copies; deferred 3 iterations off the critical path
  - epilogue: ACT Prelu(psum+bias, alpha=.01) IN PLACE on psum (same ACT
    table as sigmoid -> no table reloads) -> ACT Sigmoid -> bf16 sg tile.
    Output is ONE channel per timestep ([T, H, W]); the host broadcasts to
    the 3 channels (reference broadcasts before the elementwise sigmoid, so
    results are identical) -> output DMA volume and sigmoid work cut 3x
  - c2 sigmoids write a persistent [8, 16, W] tile; ONE final DMA stores
    rows 248..255 for all timesteps
  - schedule: loads 3 groups ahead (wait-free gpsimd software-DGE triggers),
    pools 2 ahead of convs, output triggers age 2 checkpoints so no gpsimd
    trigger ever blocks on an incomplete producer; group 0 is loaded and
    pooled per-sub with dedicated tiles so PE starts ~8us into the run
"""
import sys

sys.path.insert(0, "/opt/trn_rl_repo")

import numpy as np
import ml_dtypes
from contextlib import ExitStack

import concourse.bass as bass
import concourse.tile as tile
from concourse import bacc, mybir
from concourse.bass_utils import run_bass_kernel_spmd

F32 = mybir.dt.float32
BF16 = mybir.dt.bfloat16

B, CTOT, H, W = 8, 48, 256, 256
T = 16
N_CORES = 8
NGRP = 8            # groups of 6 planes = 2 timesteps
GP = 6              # planes per group
# main chunk geometry: y rows [m0,m1) from x rows [r0,r1)
SUBS = [(0, 124, 0, 128), (124, 248, 122, 250)]
C2 = (248, 256, 246, 256)   # last-8-rows chunk
NMAIN = 2 * 3 * 3 * 2       # path, c, kw, sub
NC2 = 2 * 3                 # path, c (kw folded into K=30)
NAVG5 = 3 * 5 * 2           # c, s, sub (avg path as 5-tap on X, no bh pools)
NMAT = NMAIN + NC2 + NAVG5
# groups whose avg path runs on PE as 5 horizontal taps against X directly
# (skipping the bh DVE pools) — balances DVE (bottleneck) against PE slack
MOVED = (0, 1, 2, 3)
AVG5_TAPS = (-2, -1, 1, 2, 0)   # s=0 last: the stop matmul is full width

_cache = {}


def _build_stack(conv_w):
    """lhsT stack [128, NMAT, 128] bf16.

    mats 0..35: main-sub ops, idx = ((path*3 + c)*3 + kw)*2 + sub,
      lhsT = op[m0:m1, r0:r1].T  ([K=128, M=124])
    mats 36..41: chunk-2 stacked ops, idx = 36 + path*3 + c,
      [K=30, M=8]: K blocks of 10 rows for kw = 1 (center), 0, 2,
      replicated at partition bases 0/32/64/96.
    """
    w = conv_w[0].astype(np.float64)  # [6, 3, 3]
    Bv = np.zeros((H, H))
    for i in (-1, 0, 1):
        Bv += np.eye(H, k=i)
    stack = np.zeros((128, NMAT, 128), dtype=np.float64)

    def band_op(path, c, kw):
        op = np.zeros((H, H))
        k2d = w[2 * c] if path == 0 else w[2 * c + 1]
        for kh in range(3):
            op += k2d[kh, kw] * np.eye(H, k=kh - 1)
        if path == 1:
            op = (op @ Bv) / 9.0
        return op

    def band_op5(c, s):
        """Avg path folded horizontally: 5-tap effective kernel e[s] =
        (w_avg[kh, :] * [1,1,1])[s]; vertical box stays in the band."""
        op = np.zeros((H, H))
        k2d = w[2 * c + 1]
        for kh in range(3):
            e = 0.0
            for kw in range(3):
                if abs(kw - 1 - s) <= 1:
                    e += k2d[kh, kw]
            op += e * np.eye(H, k=kh - 1)
        return (op @ Bv) / 9.0

    for path in range(2):
        for c in range(3):
            for kw in range(3):
                op = band_op(path, c, kw)
                for sub, (m0, m1, r0, r1) in enumerate(SUBS):
                    mat = ((path * 3 + c) * 3 + kw) * 2 + sub
                    lhsT = op[m0:m1, r0:r1].T  # [K, M]
                    K, M = lhsT.shape
                    stack[:K, mat, :M] = lhsT
            # chunk 2: kw-stacked [30, 8]
            mat = NMAIN + path * 3 + c
            m0, m1, r0, r1 = C2
            for kwi, kw in enumerate((1, 0, 2)):
                lhsT = band_op(path, c, kw)[m0:m1, r0:r1].T  # [10, 8]
                for base in (0, 32, 64, 96):
                    stack[base + 10 * kwi:base + 10 * kwi + 10, mat, :8] = lhsT
    for c in range(3):
        for si, s in enumerate(AVG5_TAPS):
            op = band_op5(c, s)
            for sub, (m0, m1, r0, r1) in enumerate(SUBS):
                mat = NMAIN + NC2 + (c * 5 + si) * 2 + sub
                lhsT = op[m0:m1, r0:r1].T
                K, M = lhsT.shape
                stack[:K, mat, :M] = lhsT
    return stack.astype(ml_dtypes.bfloat16)


def _mat_main(path, c, kw, sub):
    return ((path * 3 + c) * 3 + kw) * 2 + sub


def _mat_c2(path, c):
    return NMAIN + path * 3 + c


def _mat_avg5(c, si, sub):
    return NMAIN + NC2 + (c * 5 + si) * 2 + sub


def _prep_x(xi):
    """Host-side re-layout: [plane, row, w] f32 -> [row+pad, plane, w] bf16.

    Row r of the padded tensor holds image row r-1; rows 0 and 257 replicate
    the image edge rows (the maxpool clamp; conv coefficients there are 0).
    With rows outermost, every on-chip tile load is a contiguous HBM read per
    partition, and the row-shifted slices are offset views of the same rows.
    """
    xp = np.empty((H + 2, CTOT, W), dtype=ml_dtypes.bfloat16)
    xp[1:H + 1] = xi.transpose(1, 0, 2)
    xp[0] = xp[1]
    xp[H + 1] = xp[H]
    return xp


def _build_program():
    nc = bacc.Bacc("TRN2", target_bir_lowering=False, debug=False, enable_asserts=False)
    x_ap = nc.dram_tensor("x", [H + 2, CTOT, W], BF16, kind="ExternalInput").ap()
    cst_ap = nc.dram_tensor("cst", [128, NMAT, 128], BF16, kind="ExternalInput").ap()
    bias_ap = nc.dram_tensor("bias", [128, 1], F32, kind="ExternalInput").ap()
    # bf16, one channel per timestep: sigmoid outputs lie in (0,1) so bf16
    # quantization (~0.4% rel) is far inside the accuracy budget; the host
    # upcasts to f32 and broadcasts each timestep mask to its 3 channels.
    out_ap = nc.dram_tensor("out", [T, H, W], BF16, kind="ExternalOutput").ap()

    MAXOP = mybir.AluOpType.max
    ADDOP = mybir.AluOpType.add
    RWST = CTOT * W          # HBM row stride (elements)

    with tile.TileContext(nc) as tc, ExitStack() as ctx:
        const_pool = ctx.enter_context(tc.tile_pool(name="const", bufs=1))
        psum_pool = ctx.enter_context(tc.tile_pool(name="psum", bufs=6, space="PSUM"))
        sg_pool = ctx.enter_context(tc.tile_pool(name="sg", bufs=6))
        t2_pool = ctx.enter_context(tc.tile_pool(name="t2", bufs=1))
        # 4 x bufs: load_xud(g+3) at iteration g then recycles the buffer of
        # group g-1, whose conv readers (MOVED groups read X as the avg-path
        # rhs) were issued at iteration g-1 — program order stays consistent
        x_pool = ctx.enter_context(tc.tile_pool(name="xload", bufs=4))
        mxbh_pool = ctx.enter_context(tc.tile_pool(name="mxbh", bufs=3))
        g0_pool = ctx.enter_context(tc.tile_pool(name="g0", bufs=1))

        cst = const_pool.tile([128, NMAT, 128], BF16, tag="cst")
        bias = const_pool.tile([128, 1], F32, tag="bias")
        # (cst/bias DMAs are issued AFTER the first input loads: cst is only
        # needed by the first matmul ~13us in, while the input loads gate the
        # DVE pools — the 1.4MB cst transfer must not hog the DMA engines
        # during the first microseconds)

        # ---- t2 tiles: rows 246..255 of quadrant q (planes 12q..12q+11) at
        # partitions 32q..32q+9. MX/BH are padded to 258 cols (data at cols
        # 1..256, zero pads) and also hold R/L column-shifted copies at
        # partition offsets +10 / +20 (kw-folded K=30).
        WP = W + 2
        QP = 12  # planes per t2 quadrant (2 groups)
        T2X = t2_pool.tile([128, QP, W], BF16, tag="t2x")
        T2U = t2_pool.tile([128, QP, W], BF16, tag="t2u")
        T2D = t2_pool.tile([128, QP, W], BF16, tag="t2d")
        T2MX = t2_pool.tile([128, QP, WP], BF16, tag="t2mx")
        T2BH = t2_pool.tile([128, QP, WP], BF16, tag="t2bh")
        # c2 sigmoid accumulator: rows 248..255 x all 16 timesteps; ONE
        # final DMA stores it
        C2OUT = t2_pool.tile([8, T, W], BF16, tag="c2out")
        # (no full-tile zeroing: garbage in gap partitions only flows into
        # regions later overwritten by the shift DMAs or never read; the
        # pad columns that ARE read get strided memsets in t2_pools)

        # ---- fused input load: tile [128, 2 sub, 3 shift, GP, W]; shift j
        # holds padded rows (r0_sub + p + j) so the vertical 3-max is three
        # aligned slices of ONE tile. The HBM source AP repeats the row
        # stride across the shift dim (flat DRAM addressing): one software
        # DGE trigger per group.
        xud_tiles = {}

        def _src_ap(g, subs=(0, 1)):
            dims = [[RWST, 128]]
            if len(subs) == 2:
                dims.append([SUBS[1][2] * RWST, 2])
            dims += [[RWST, 3], [W, GP], [1, W]]
            off = GP * g * W + SUBS[subs[0]][2] * RWST
            return bass.AP(x_ap.tensor, off, dims)

        def load_xud(g):
            X = x_pool.tile([128, 2, 3, GP, W], BF16, tag="x")
            for sub in range(2):
                nc.gpsimd.dma_start(out=X[:, sub:sub + 1], in_=_src_ap(g, (sub,)))
            xud_tiles[g] = X

        def pools(g, X=None, mx=None, bh=None, sub=None):
            """DVE pools; when sub is given, operate on that sub slice only
            (used for group 0's fast start with dedicated tiles). Groups in
            MOVED skip the bh pools (their avg path runs on PE against X)."""
            moved = g in MOVED
            if X is None:
                X = xud_tiles.pop(g)
            if mx is None:
                mx = mxbh_pool.tile([128, 2, GP, WP], BF16, tag="mx")
                bh = None if moved else mxbh_pool.tile([128, 2, GP, WP], BF16,
                                                       tag="bh")
            s = slice(None) if sub is None else slice(sub, sub + 1)
            D, XC, U = X[:, s, 0], X[:, s, 1], X[:, s, 2]
            # one strided memset zeroes both pad columns (0 and 257)
            nc.vector.memset(mx[:, s, :, 0:258:257], 0)
            # vertical 3-row max (DVE), in place into the U slice
            vx = U
            nc.vector.tensor_tensor(out=vx, in0=U, in1=D, op=MAXOP)
            nc.vector.tensor_tensor(out=vx, in0=vx, in1=XC, op=MAXOP)
            # horizontal 3-tap max (DVE) into padded mx
            nc.vector.tensor_tensor(out=mx[:, s, :, 1:256], in0=vx[:, :, :, 0:255],
                                    in1=vx[:, :, :, 1:256], op=MAXOP)
            nc.vector.tensor_copy(mx[:, s, :, 256:257], vx[:, :, :, 255:256])
            nc.vector.tensor_tensor(out=mx[:, s, :, 2:257], in0=mx[:, s, :, 2:257],
                                    in1=vx[:, :, :, 0:255], op=MAXOP)
            if not moved:
                # horizontal 3-tap box sum (DVE) into padded bh
                nc.vector.memset(bh[:, s, :, 0:258:257], 0)
                nc.vector.tensor_tensor(out=bh[:, s, :, 1:256],
                                        in0=XC[:, :, :, 0:255],
                                        in1=XC[:, :, :, 1:256], op=ADDOP)
                nc.vector.tensor_copy(bh[:, s, :, 256:257], XC[:, :, :, 255:256])
                nc.vector.tensor_tensor(out=bh[:, s, :, 2:257],
                                        in0=bh[:, s, :, 2:257],
                                        in1=XC[:, :, :, 0:255], op=ADDOP)
            return mx, bh, X

        def load_t2(q):
            b = 32 * q
            m0, m1, r0, r1 = C2
            p0, p1 = QP * q, QP * q + QP
            nc.sync.dma_start(out=T2X[b:b + 10], in_=x_ap[r0 + 1:r1 + 1, p0:p1, :])
            nc.sync.dma_start(out=T2U[b:b + 10], in_=x_ap[r0 + 2:r1 + 2, p0:p1, :])
            nc.sync.dma_start(out=T2D[b:b + 10], in_=x_ap[r0:r1, p0:p1, :])

        def _t2_shifts(tl):
            # R/L column-shifted copies into partition blocks +10 / +20.
            # All matmul rhs windows read cols 1..256 of their block:
            #   block +10 pairs kw=0 (needs P[w-1]): dest col j <- data col j-1
            #   block +20 pairs kw=2 (needs P[w+1]): dest col j <- data col j+1
            # The widened [0:256]/[2:258] sources carry the zero pad edge.
            for q in range(4):
                b = 32 * q
                nc.sync.dma_start(out=tl[b + 10:b + 20, :, 1:257],
                                  in_=tl[b:b + 10, :, 0:256])
                nc.sync.dma_start(out=tl[b + 20:b + 30, :, 1:257],
                                  in_=tl[b:b + 10, :, 2:258])

        def t2_pools(chunk):
            """Pools over the packed t2 tile, issued in 3 chunks so the DVE
            bursts interleave between main-group pools instead of stalling
            a whole group's worth of PE work."""
            vx = T2U
            if chunk == 0:
                nc.vector.tensor_tensor(out=vx[:], in0=T2U[:], in1=T2D[:],
                                        op=MAXOP)
                nc.vector.tensor_tensor(out=vx[:], in0=vx[:], in1=T2X[:],
                                        op=MAXOP)
                nc.vector.memset(T2MX[:, :, 0:258:257], 0)
                nc.vector.memset(T2BH[:, :, 0:258:257], 0)
            elif chunk == 1:
                nc.vector.tensor_tensor(out=T2MX[:, :, 1:256], in0=vx[:, :, 0:255],
                                        in1=vx[:, :, 1:256], op=MAXOP)
                nc.vector.tensor_copy(T2MX[:, :, 256:257], vx[:, :, 255:256])
                nc.vector.tensor_tensor(out=T2MX[:, :, 2:257],
                                        in0=T2MX[:, :, 2:257],
                                        in1=vx[:, :, 0:255], op=MAXOP)
                _t2_shifts(T2MX)
            else:
                nc.vector.tensor_tensor(out=T2BH[:, :, 1:256], in0=T2X[:, :, 0:255],
                                        in1=T2X[:, :, 1:256], op=ADDOP)
                nc.vector.tensor_copy(T2BH[:, :, 256:257], T2X[:, :, 255:256])
                nc.vector.tensor_tensor(out=T2BH[:, :, 2:257],
                                        in0=T2BH[:, :, 2:257],
                                        in1=T2X[:, :, 0:255], op=ADDOP)
                _t2_shifts(T2BH)

        out_ready = []   # sigmoids surely complete: safe to issue triggers
        out_recent = []  # freshly issued sigmoids: age one checkpoint first

        def epilogue_lrelu(ps, M):
            """ACT Prelu(psum + bias) IN PLACE on the psum bank. Prelu
            (parametric_relu) lives in the same ACT function table as
            sigmoid, so alternating them costs no ACT_TABLE_LOADs."""
            nc.scalar.activation(ps[0:M], ps[0:M],
                                 mybir.ActivationFunctionType.Prelu,
                                 bias=bias[0:M], scale=1.0, alpha=0.01)
            return ps

        def epilogue_sigmoid(ps, M, t0, m0, m1):
            """Sigmoid psum -> bf16 sg (one channel per timestep). Output
            DMA issue is DEFERRED (gpsimd queue is in-order: a trigger
            waiting on its sigmoid would stall later load triggers)."""
            sg = sg_pool.tile([128, 2, W], BF16, tag="epis")
            nc.scalar.activation(sg[0:M], ps[0:M],
                                 mybir.ActivationFunctionType.Sigmoid)
            out_recent.append((sg, M, t0, m0, m1))

        def flush_outputs(final=False):
            for sg, M, t0, m0, m1 in out_ready:
                dst = out_ap[t0:t0 + 2, m0:m1, :].transpose([1, 0, 2])
                nc.gpsimd.dma_start(out=dst, in_=sg[0:M])
            out_ready.clear()
            out_ready.extend(out_recent)
            out_recent.clear()
            if final and out_ready:
                flush_outputs()

        def conv_sub(g, sub, mx, bh, X):
            """Main-chunk accumulation for group g's timestep pair.

            Max-path matmuls are full-width N=512 against the padded mx
            (whose zero pads supply the conv zero padding). For MOVED
            groups the avg path is 5 horizontal taps against X directly:
            clipped rhs windows accumulate into column-offset psum slices,
            so unwritten psum columns realize the conv zero padding.
            """
            m0, m1, r0, r1 = SUBS[sub]
            M, K = m1 - m0, r1 - r0
            moved = g in MOVED
            sb = 0 if mx.shape[1] == 1 else sub
            ps = psum_pool.tile([128, 2, W], F32, tag="ps")
            i, n = 0, 24 if moved else 18
            for c in range(3):
                for kw in (1, 0, 2):
                    s = kw - 1
                    mat = _mat_main(0, c, kw, sub)
                    rhs = mx[0:K, sb, c:c + 4:3, 1 + s:257 + s]
                    nc.tensor.matmul(ps[0:M], cst[0:K, mat, 0:M], rhs,
                                     start=(i == 0), stop=(i == n - 1))
                    i += 1
            if moved:
                XC = X[:, sb, 1]
                for si, s in enumerate(AVG5_TAPS):
                    a, b = max(0, s), W + min(0, s)
                    for c in range(3):
                        mat = _mat_avg5(c, si, sub)
                        rhs = XC[0:K, c:c + 4:3, a:b]
                        nc.tensor.matmul(ps[0:M, :, a - s:b - s],
                                         cst[0:K, mat, 0:M], rhs,
                                         start=False, stop=(i == n - 1))
                        i += 1
            else:
                for c in range(3):
                    for kw in (1, 0, 2):
                        s = kw - 1
                        mat = _mat_main(1, c, kw, sub)
                        rhs = bh[0:K, sb, c:c + 4:3, 1 + s:257 + s]
                        nc.tensor.matmul(ps[0:M], cst[0:K, mat, 0:M], rhs,
                                         start=False, stop=(i == n - 1))
                        i += 1
            epilogue_lrelu(ps, M)
            return ps, M, 2 * g, m0, m1

        def conv_c2(g):
            """Last-8-rows accumulation (kw-folded, K=30) for group g; the
            sigmoid lands in the persistent C2OUT tile."""
            m0, m1, r0, r1 = C2
            M = m1 - m0
            b = 32 * (g // 2)
            pb = 6 * (g % 2)
            ps = psum_pool.tile([128, 2, W], F32, tag="ps")
            idx = 0
            for path in range(2):
                for c in range(3):
                    mat = _mat_c2(path, c)
                    src = T2MX if path == 0 else T2BH
                    rhs = src[b:b + 30, pb + c:pb + c + 4:3, 1:257]
                    nc.tensor.matmul(ps[0:M], cst[b:b + 30, mat, 0:M], rhs,
                                     start=(idx == 0), stop=(idx == NC2 - 1),
                                     tile_position=(b, 0))
                    idx += 1
            epilogue_lrelu(ps, M)
            nc.scalar.activation(C2OUT[0:M, 2 * g:2 * g + 2], ps[0:M],
                                 mybir.ActivationFunctionType.Sigmoid)

        # ---- schedule: group 0 is loaded per-sub with dedicated pool tiles
        # so the first matmul only waits on sub 0's load + 6 DVE ops. Later
        # loads run 3 groups ahead; pools for g+2 are issued at the top of
        # iteration g so they execute while PE runs group g's convs. Output
        # triggers age through two checkpoints before issue.
        # conv_c2(g) is deferred three iterations (it only needs the t2 pools
        # and a psum bank), so t2 pool work stays off the early critical path
        g0X = x_pool.tile([128, 2, 3, GP, W], BF16, tag="x")
        g0t = []
        for sub in range(2):
            nc.gpsimd.dma_start(out=g0X[:, sub:sub + 1], in_=_src_ap(0, (sub,)))
            mxs = g0_pool.tile([128, 1, GP, WP], BF16, tag=f"g0mx{sub}")
            mxs, _, _ = pools(0, X=g0X[:, sub:sub + 1], mx=mxs, bh=None, sub=0)
            g0t.append((mxs, None, g0X[:, sub:sub + 1]))
        for g in (1, 2):
            load_xud(g)
        nc.sync.dma_start(out=cst[:], in_=cst_ap)
        nc.sync.dma_start(out=bias[:], in_=bias_ap)
        pools_of = {1: pools(1), 2: pools(2)}
        for q in range(4):
            load_t2(q)
        # c2 deferral: conv_c2(g) runs 4 iterations later, after the t2
        # pool chunks (issued at g=1..3 between main pools) are done
        for g in range(NGRP):
            if g + 3 < NGRP:
                load_xud(g + 3)
            if g + 2 < NGRP and g >= 1:
                pools_of[g + 2] = pools(g + 2)
            if g == 0:
                epilogue_sigmoid(*conv_sub(0, 0, *g0t[0]))
                epilogue_sigmoid(*conv_sub(0, 1, *g0t[1]))
            else:
                mx, bh, X = pools_of.pop(g)
                epilogue_sigmoid(*conv_sub(g, 0, mx, bh, X))
                epilogue_sigmoid(*conv_sub(g, 1, mx, bh, X))
            if g in (1, 2, 3):
                t2_pools(g - 1)
            flush_outputs()
            if g >= 4:
                conv_c2(g - 4)
        for g in (NGRP - 4, NGRP - 3, NGRP - 2, NGRP - 1):
            conv_c2(g)
        flush_outputs(final=True)
        # one DMA for all last-8 rows: [T, 8, W] <- C2OUT[0:8] transposed
        m0 = C2[0]
        nc.sync.dma_start(out=out_ap[:, m0:m0 + 8, :].transpose([1, 0, 2]),
                          in_=C2OUT[0:8])

    nc.compile()
    return nc


def kernel(input_tensor, conv_w, conv_b):
    input_tensor = np.ascontiguousarray(np.asarray(input_tensor, dtype=np.float32))
    conv_w = np.asarray(conv_w, dtype=np.float32)
    conv_b = np.asarray(conv_b, dtype=np.float32)

    if "nc" not in _cache:
        _cache["nc"] = _build_program()
    nc = _cache["nc"]

    stack = _build_stack(conv_w)
    bias_vec = np.full((128, 1), conv_b[0], dtype=np.float32)
    in_maps = [
        {"x": _prep_x(input_tensor[i]), "cst": stack, "bias": bias_vec}
        for i in range(N_CORES)
    ]
    res = run_bass_kernel_spmd(nc, in_maps, list(range(N_CORES)))
    # [T, H, W] bf16 per core -> broadcast each timestep mask to 3 channels
    out = np.stack([res.results[i]["out"] for i in range(N_CORES)], axis=0)
    out = np.repeat(out.astype(np.float32), 3, axis=1)
    return out


if __name__ == "__main__":
    rng = np.random.default_rng(0)
    x = rng.standard_normal((B, CTOT, H, W), dtype=np.float32)
    cw = rng.uniform(-0.1, 0.1, (1, 6, 3, 3)).astype(np.float32)
    cb = np.array([0.01], dtype=np.float32)
    o = kernel(x, cw, cb)
    print(o.shape, o.dtype)


# revision 22
# speedup vs baseline: 1.1296x; 1.0274x over previous
"""Trainium2 Bass kernel for nn_CBAMSpaceMask (CBAM spatial mask over T timestep blocks).

Math per timestep block t (3 channels):
  mx_c = maxpool3x3(x_c)          (stride 1, -inf pad == replicate pad)
  av_c = avgpool3x3(x_c)/9        (zero pad, count_include_pad)
  y_t  = sum_c wM_c * mx_c + wA_c * av_c + b   (3x3 conv, zero pad)
  out[3t+c] = sigmoid(leakyrelu(y_t))          (broadcast over c)

Design (per core = 1 batch element, pure data parallel over batch):
  - host-side re-layout: input -> [row+2pad, plane, w] bf16; pad rows replicate
    image edges (maxpool -inf-pad clamp; conv coefficients there are zero)
  - ONE overlapping-AP DMA per group loads a [128, 2sub, 3shift, 6, W] tile:
    shift j supplies row p+j-1 on partition p (HBM APs are flat, so the row
    stride can repeat across the shift dim); this replaces 3 separate X/U/D
    loads per sub (1 gpsimd software-DGE trigger per group instead of 6)
  - groups of 6 planes (= 2 timesteps, one matmul pair); both row-subs
    (y rows 0:124 and 124:248) share the tile
  - pools: vertical 3-row max (2 DVE ops over the shift slices), horizontal
    3-tap max and box sum (2 DVE ops each) into 258-col zero-padded mx/bh;
    vertical box sum of the avg path folded into the conv operator (op@Bv);
    pad-column zeroing via ONE strided-AP memset per tile (cols 0 and 257)
  - conv: banded-Toeplitz matmuls on PE; timestep-paired rhs (planes
    {c, c+3} via stride-3 slice) -> all matmuls full-width N=512, padded
    windows supply the conv zero padding; psum [124, 2, 256]; max-path
    matmuls issued first so the bh pools may lag the mx pools
  - last-8-rows chunk: rows 246..255 packed per quadrant at partition
    bases 0/32/64/96; kw taps folded into K=30 stacked matmuls using R/L
    column-shifted copies; deferred 3 iterations off the critical path
  - epilogue: ACT Prelu(psum+bias, alpha=.01) IN PLACE on psum (same ACT
    table as sigmoid -> no table reloads) -> ACT Sigmoid -> bf16 sg tile.
    Output is ONE channel per timestep ([T, H, W]); the host broadcasts to
    the 3 channels (reference broadcasts before the elementwise sigmoid, so
    results are identical) -> output DMA volume and sigmoid work cut 3x
  - c2 sigmoids write a persistent [8, 16, W] tile; ONE final DMA stores
    rows 248..255 for all timesteps
  - schedule: loads 3 groups ahead (wait-free gpsimd software-DGE triggers),
    pools 2 ahead of convs, output triggers age 2 checkpoints so no gpsimd
    trigger ever blocks on an incomplete producer; group 0 is loaded and
    pooled per-sub with dedicated tiles so PE starts ~8us into the run
"""
import sys

sys.path.insert(0, "/opt/trn_rl_repo")

import numpy as np
import ml_dtypes
from contextlib import ExitStack

import concourse.bass as bass
import concourse.tile as tile
from concourse import bacc, mybir
from concourse.bass_utils import run_bass_kernel_spmd

F32 = mybir.dt.float32
BF16 = mybir.dt.bfloat16

B, CTOT, H, W = 8, 48, 256, 256
T = 16
N_CORES = 8
NGRP = 8            # groups of 6 planes = 2 timesteps
GP = 6              # planes per group
# main chunk geometry: y rows [m0,m1) from x rows [r0,r1)
SUBS = [(0, 124, 0, 128), (124, 248, 122, 250)]
C2 = (248, 256, 246, 256)   # last-8-rows chunk
NMAIN = 2 * 3 * 3 * 2       # path, c, kw, sub
NC2 = 2 * 3                 # path, c (kw folded into K=30)
NAVG5 = 3 * 5 * 2           # c, s, sub (avg path as 5-tap on X, no bh pools)
NMAT = NMAIN + NC2 + NAVG5
# groups whose avg path runs on PE as 5 horizontal taps against X directly
# (skipping the bh DVE pools) — balances DVE (bottleneck) against PE slack
MOVED = (1, 2)
AVG5_TAPS = (-2, -1, 1, 2, 0)   # s=0 last: the stop matmul is full width

_cache = {}


def _build_stack(conv_w):
    """lhsT stack [128, NMAT, 128] bf16.

    mats 0..35: main-sub ops, idx = ((path*3 + c)*3 + kw)*2 + sub,
      lhsT = op[m0:m1, r0:r1].T  ([K=128, M=124])
    mats 36..41: chunk-2 stacked ops, idx = 36 + path*3 + c,
      [K=30, M=8]: K blocks of 10 rows for kw = 1 (center), 0, 2,
      replicated at partition bases 0/32/64/96.
    """
    w = conv_w[0].astype(np.float64)  # [6, 3, 3]
    Bv = np.zeros((H, H))
    for i in (-1, 0, 1):
        Bv += np.eye(H, k=i)
    stack = np.zeros((128, NMAT, 128), dtype=np.float64)

    def band_op(path, c, kw):
        op = np.zeros((H, H))
        k2d = w[2 * c] if path == 0 else w[2 * c + 1]
        for kh in range(3):
            op += k2d[kh, kw] * np.eye(H, k=kh - 1)
        if path == 1:
            op = (op @ Bv) / 9.0
        return op

    def band_op5(c, s):
        """Avg path folded horizontally: 5-tap effective kernel e[s] =
        (w_avg[kh, :] * [1,1,1])[s]; vertical box stays in the band."""
        op = np.zeros((H, H))
        k2d = w[2 * c + 1]
        for kh in range(3):
            e = 0.0
            for kw in range(3):
                if abs(kw - 1 - s) <= 1:
                    e += k2d[kh, kw]
            op += e * np.eye(H, k=kh - 1)
        return (op @ Bv) / 9.0

    for path in range(2):
        for c in range(3):
            for kw in range(3):
                op = band_op(path, c, kw)
                for sub, (m0, m1, r0, r1) in enumerate(SUBS):
                    mat = ((path * 3 + c) * 3 + kw) * 2 + sub
                    lhsT = op[m0:m1, r0:r1].T  # [K, M]
                    K, M = lhsT.shape
                    stack[:K, mat, :M] = lhsT
            # chunk 2: kw-stacked [30, 8]
            mat = NMAIN + path * 3 + c
            m0, m1, r0, r1 = C2
            for kwi, kw in enumerate((1, 0, 2)):
                lhsT = band_op(path, c, kw)[m0:m1, r0:r1].T  # [10, 8]
                for base in (0, 32, 64, 96):
                    stack[base + 10 * kwi:base + 10 * kwi + 10, mat, :8] = lhsT
    for c in range(3):
        for si, s in enumerate(AVG5_TAPS):
            op = band_op5(c, s)
            for sub, (m0, m1, r0, r1) in enumerate(SUBS):
                mat = NMAIN + NC2 + (c * 5 + si) * 2 + sub
                lhsT = op[m0:m1, r0:r1].T
                K, M = lhsT.shape
                stack[:K, mat, :M] = lhsT
    return stack.astype(ml_dtypes.bfloat16)


def _mat_main(path, c, kw, sub):
    return ((path * 3 + c) * 3 + kw) * 2 + sub


def _mat_c2(path, c):
    return NMAIN + path * 3 + c


def _mat_avg5(c, si, sub):
    return NMAIN + NC2 + (c * 5 + si) * 2 + sub


def _prep_x(xi):
    """Host-side re-layout: [plane, row, w] f32 -> [row+pad, plane, w] bf16.

    Row r of the padded tensor holds image row r-1; rows 0 and 257 replicate
    the image edge rows (the maxpool clamp; conv coefficients there are 0).
    With rows outermost, every on-chip tile load is a contiguous HBM read per
    partition, and the row-shifted slices are offset views of the same rows.
    """
    xp = np.empty((H + 2, CTOT, W), dtype=ml_dtypes.bfloat16)
    xp[1:H + 1] = xi.transpose(1, 0, 2)
    xp[0] = xp[1]
    xp[H + 1] = xp[H]
    return xp


def _build_program():
    nc = bacc.Bacc("TRN2", target_bir_lowering=False, debug=False, enable_asserts=False)
    x_ap = nc.dram_tensor("x", [H + 2, CTOT, W], BF16, kind="ExternalInput").ap()
    cst_ap = nc.dram_tensor("cst", [128, NMAT, 128], BF16, kind="ExternalInput").ap()
    bias_ap = nc.dram_tensor("bias", [128, 1], F32, kind="ExternalInput").ap()
    # bf16, one channel per timestep: sigmoid outputs lie in (0,1) so bf16
    # quantization (~0.4% rel) is far inside the accuracy budget; the host
    # upcasts to f32 and broadcasts each timestep mask to its 3 channels.
    out_ap = nc.dram_tensor("out", [T, H, W], BF16, kind="ExternalOutput").ap()

    MAXOP = mybir.AluOpType.max
    ADDOP = mybir.AluOpType.add
    RWST = CTOT * W          # HBM row stride (elements)

    with tile.TileContext(nc) as tc, ExitStack() as ctx:
        const_pool = ctx.enter_context(tc.tile_pool(name="const", bufs=1))
        psum_pool = ctx.enter_context(tc.tile_pool(name="psum", bufs=6, space="PSUM"))
        sg_pool = ctx.enter_context(tc.tile_pool(name="sg", bufs=6))
        t2_pool = ctx.enter_context(tc.tile_pool(name="t2", bufs=1))
        # 4 x bufs: load_xud(g+3) at iteration g then recycles the buffer of
        # group g-1, whose conv readers (MOVED groups read X as the avg-path
        # rhs) were issued at iteration g-1 — program order stays consistent
        x_pool = ctx.enter_context(tc.tile_pool(name="xload", bufs=4))
        mxbh_pool = ctx.enter_context(tc.tile_pool(name="mxbh", bufs=3))
        g0_pool = ctx.enter_context(tc.tile_pool(name="g0", bufs=1))

        cst = const_pool.tile([128, NMAT, 128], BF16, tag="cst")
        bias = const_pool.tile([128, 1], F32, tag="bias")
        # (cst/bias DMAs are issued AFTER the first input loads: cst is only
        # needed by the first matmul ~13us in, while the input loads gate the
        # DVE pools — the 1.4MB cst transfer must not hog the DMA engines
        # during the first microseconds)

        # ---- t2 tiles: rows 246..255 of quadrant q (planes 12q..12q+11) at
        # partitions 32q..32q+9. MX/BH are padded to 258 cols (data at cols
        # 1..256, zero pads) and also hold R/L column-shifted copies at
        # partition offsets +10 / +20 (kw-folded K=30).
        WP = W + 2
        QP = 12  # planes per t2 quadrant (2 groups)
        T2X = t2_pool.tile([128, QP, W], BF16, tag="t2x")
        T2U = t2_pool.tile([128, QP, W], BF16, tag="t2u")
        T2D = t2_pool.tile([128, QP, W], BF16, tag="t2d")
        T2MX = t2_pool.tile([128, QP, WP], BF16, tag="t2mx")
        T2BH = t2_pool.tile([128, QP, WP], BF16, tag="t2bh")
        # c2 sigmoid accumulator: rows 248..255 x all 16 timesteps; ONE
        # final DMA stores it
        C2OUT = t2_pool.tile([8, T, W], BF16, tag="c2out")
        # (no full-tile zeroing: garbage in gap partitions only flows into
        # regions later overwritten by the shift DMAs or never read; the
        # pad columns that ARE read get strided memsets in t2_pools)

        # ---- fused input load: tile [128, 2 sub, 3 shift, GP, W]; shift j
        # holds padded rows (r0_sub + p + j) so the vertical 3-max is three
        # aligned slices of ONE tile. The HBM source AP repeats the row
        # stride across the shift dim (flat DRAM addressing): one software
        # DGE trigger per group.
        xud_tiles = {}

        def _src_ap(g, subs=(0, 1)):
            dims = [[RWST, 128]]
            if len(subs) == 2:
                dims.append([SUBS[1][2] * RWST, 2])
            dims += [[RWST, 3], [W, GP], [1, W]]
            off = GP * g * W + SUBS[subs[0]][2] * RWST
            return bass.AP(x_ap.tensor, off, dims)

        def load_xud(g):
            X = x_pool.tile([128, 2, 3, GP, W], BF16, tag="x")
            for sub in range(2):
                nc.gpsimd.dma_start(out=X[:, sub:sub + 1], in_=_src_ap(g, (sub,)))
            xud_tiles[g] = X

        def pools(g, X=None, mx=None, bh=None, sub=None):
            """DVE pools; when sub is given, operate on that sub slice only
            (used for group 0's fast start with dedicated tiles). Groups in
            MOVED skip the bh pools (their avg path runs on PE against X)."""
            moved = g in MOVED
            if X is None:
                X = xud_tiles.pop(g)
            if mx is None:
                mx = mxbh_pool.tile([128, 2, GP, WP], BF16, tag="mx")
                bh = None if moved else mxbh_pool.tile([128, 2, GP, WP], BF16,
                                                       tag="bh")
            s = slice(None) if sub is None else slice(sub, sub + 1)
            D, XC, U = X[:, s, 0], X[:, s, 1], X[:, s, 2]
            # one strided memset zeroes both pad columns (0 and 257)
            nc.vector.memset(mx[:, s, :, 0:258:257], 0)
            # vertical 3-row max (DVE), in place into the U slice
            vx = U
            nc.vector.tensor_tensor(out=vx, in0=U, in1=D, op=MAXOP)
            nc.vector.tensor_tensor(out=vx, in0=vx, in1=XC, op=MAXOP)
            # horizontal 3-tap max (DVE) into padded mx
            nc.vector.tensor_tensor(out=mx[:, s, :, 1:256], in0=vx[:, :, :, 0:255],
                                    in1=vx[:, :, :, 1:256], op=MAXOP)
            nc.vector.tensor_copy(mx[:, s, :, 256:257], vx[:, :, :, 255:256])
            nc.vector.tensor_tensor(out=mx[:, s, :, 2:257], in0=mx[:, s, :, 2:257],
                                    in1=vx[:, :, :, 0:255], op=MAXOP)
            if not moved:
                # horizontal 3-tap box sum (DVE) into padded bh
                nc.vector.memset(bh[:, s, :, 0:258:257], 0)
                nc.vector.tensor_tensor(out=bh[:, s, :, 1:256],
                                        in0=XC[:, :, :, 0:255],
                                        in1=XC[:, :, :, 1:256], op=ADDOP)
                nc.vector.tensor_copy(bh[:, s, :, 256:257], XC[:, :, :, 255:256])
                nc.vector.tensor_tensor(out=bh[:, s, :, 2:257],
                                        in0=bh[:, s, :, 2:257],
                                        in1=XC[:, :, :, 0:255], op=ADDOP)
            return mx, bh, X

        def load_t2(q):
            # on the gpsimd queue: early DMAs share ONE FIFO so issue order
            # is true priority order (engines drain queues round-robin, so a
            # second queue would steal bandwidth from the critical g0 load)
            b = 32 * q
            m0, m1, r0, r1 = C2
            p0, p1 = QP * q, QP * q + QP
            nc.gpsimd.dma_start(out=T2X[b:b + 10], in_=x_ap[r0 + 1:r1 + 1, p0:p1, :])
            nc.gpsimd.dma_start(out=T2U[b:b + 10], in_=x_ap[r0 + 2:r1 + 2, p0:p1, :])
            nc.gpsimd.dma_start(out=T2D[b:b + 10], in_=x_ap[r0:r1, p0:p1, :])

        def _t2_shifts(tl):
            # R/L column-shifted copies into partition blocks +10 / +20.
            # All matmul rhs windows read cols 1..256 of their block:
            #   block +10 pairs kw=0 (needs P[w-1]): dest col j <- data col j-1
            #   block +20 pairs kw=2 (needs P[w+1]): dest col j <- data col j+1
            # The widened [0:256]/[2:258] sources carry the zero pad edge.
            for q in range(4):
                b = 32 * q
                nc.sync.dma_start(out=tl[b + 10:b + 20, :, 1:257],
                                  in_=tl[b:b + 10, :, 0:256])
                nc.sync.dma_start(out=tl[b + 20:b + 30, :, 1:257],
                                  in_=tl[b:b + 10, :, 2:258])

        def t2_pools(chunk):
            """Pools over the packed t2 tile, issued in 3 chunks so the DVE
            bursts interleave between main-group pools instead of stalling
            a whole group's worth of PE work."""
            vx = T2U
            if chunk == 0:
                nc.vector.tensor_tensor(out=vx[:], in0=T2U[:], in1=T2D[:],
                                        op=MAXOP)
                nc.vector.tensor_tensor(out=vx[:], in0=vx[:], in1=T2X[:],
                                        op=MAXOP)
                nc.vector.memset(T2MX[:, :, 0:258:257], 0)
                nc.vector.memset(T2BH[:, :, 0:258:257], 0)
            elif chunk == 1:
                nc.vector.tensor_tensor(out=T2MX[:, :, 1:256], in0=vx[:, :, 0:255],
                                        in1=vx[:, :, 1:256], op=MAXOP)
                nc.vector.tensor_copy(T2MX[:, :, 256:257], vx[:, :, 255:256])
                nc.vector.tensor_tensor(out=T2MX[:, :, 2:257],
                                        in0=T2MX[:, :, 2:257],
                                        in1=vx[:, :, 0:255], op=MAXOP)
                _t2_shifts(T2MX)
            else:
                nc.vector.tensor_tensor(out=T2BH[:, :, 1:256], in0=T2X[:, :, 0:255],
                                        in1=T2X[:, :, 1:256], op=ADDOP)
                nc.vector.tensor_copy(T2BH[:, :, 256:257], T2X[:, :, 255:256])
                nc.vector.tensor_tensor(out=T2BH[:, :, 2:257],
                                        in0=T2BH[:, :, 2:257],
                                        in1=T2X[:, :, 0:255], op=ADDOP)
                _t2_shifts(T2BH)

        out_ready = []   # sigmoids surely complete: safe to issue triggers
        out_recent = []  # freshly issued sigmoids: age one checkpoint first

        def epilogue_lrelu(ps, M):
            """ACT Prelu(psum + bias) IN PLACE on the psum bank. Prelu
            (parametric_relu) lives in the same ACT function table as
            sigmoid, so alternating them costs no ACT_TABLE_LOADs."""
            nc.scalar.activation(ps[0:M], ps[0:M],
                                 mybir.ActivationFunctionType.Prelu,
                                 bias=bias[0:M], scale=1.0, alpha=0.01)
            return ps

        def epilogue_sigmoid(ps, M, t0, m0, m1):
            """Sigmoid psum -> bf16 sg (one channel per timestep). Output
            DMA issue is DEFERRED (gpsimd queue is in-order: a trigger
            waiting on its sigmoid would stall later load triggers)."""
            sg = sg_pool.tile([128, 2, W], BF16, tag="epis")
            nc.scalar.activation(sg[0:M], ps[0:M],
                                 mybir.ActivationFunctionType.Sigmoid)
            out_recent.append((sg, M, t0, m0, m1))

        def flush_outputs(final=False):
            for sg, M, t0, m0, m1 in out_ready:
                dst = out_ap[t0:t0 + 2, m0:m1, :].transpose([1, 0, 2])
                nc.gpsimd.dma_start(out=dst, in_=sg[0:M])
            out_ready.clear()
            out_ready.extend(out_recent)
            out_recent.clear()
            if final and out_ready:
                flush_outputs()

        def conv_sub(g, sub, mx, bh, X):
            """Main-chunk accumulation for group g's timestep pair.

            Max-path matmuls are full-width N=512 against the padded mx
            (whose zero pads supply the conv zero padding). For MOVED
            groups the avg path is 5 horizontal taps against X directly:
            clipped rhs windows accumulate into column-offset psum slices,
            so unwritten psum columns realize the conv zero padding.
            """
            m0, m1, r0, r1 = SUBS[sub]
            M, K = m1 - m0, r1 - r0
            moved = g in MOVED
            sb = 0 if mx.shape[1] == 1 else sub
            ps = psum_pool.tile([128, 2, W], F32, tag="ps")
            i, n = 0, 24 if moved else 18
            for c in range(3):
                for kw in (1, 0, 2):
                    s = kw - 1
                    mat = _mat_main(0, c, kw, sub)
                    rhs = mx[0:K, sb, c:c + 4:3, 1 + s:257 + s]
                    nc.tensor.matmul(ps[0:M], cst[0:K, mat, 0:M], rhs,
                                     start=(i == 0), stop=(i == n - 1))
                    i += 1
            if moved:
                XC = X[:, sb, 1]
                for si, s in enumerate(AVG5_TAPS):
                    a, b = max(0, s), W + min(0, s)
                    for c in range(3):
                        mat = _mat_avg5(c, si, sub)
                        rhs = XC[0:K, c:c + 4:3, a:b]
                        nc.tensor.matmul(ps[0:M, :, a - s:b - s],
                                         cst[0:K, mat, 0:M], rhs,
                                         start=False, stop=(i == n - 1))
                        i += 1
            else:
                for c in range(3):
                    for kw in (1, 0, 2):
                        s = kw - 1
                        mat = _mat_main(1, c, kw, sub)
                        rhs = bh[0:K, sb, c:c + 4:3, 1 + s:257 + s]
                        nc.tensor.matmul(ps[0:M], cst[0:K, mat, 0:M], rhs,
                                         start=False, stop=(i == n - 1))
                        i += 1
            epilogue_lrelu(ps, M)
            return ps, M, 2 * g, m0, m1

        def conv_c2(g):
            """Last-8-rows accumulation (kw-folded, K=30) for group g; the
            sigmoid lands in the persistent C2OUT tile."""
            m0, m1, r0, r1 = C2
            M = m1 - m0
            b = 32 * (g // 2)
            pb = 6 * (g % 2)
            ps = psum_pool.tile([128, 2, W], F32, tag="ps")
            idx = 0
            for path in range(2):
                for c in range(3):
                    mat = _mat_c2(path, c)
                    src = T2MX if path == 0 else T2BH
                    rhs = src[b:b + 30, pb + c:pb + c + 4:3, 1:257]
                    nc.tensor.matmul(ps[0:M], cst[b:b + 30, mat, 0:M], rhs,
                                     start=(idx == 0), stop=(idx == NC2 - 1),
                                     tile_position=(b, 0))
                    idx += 1
            epilogue_lrelu(ps, M)
            nc.scalar.activation(C2OUT[0:M, 2 * g:2 * g + 2], ps[0:M],
                                 mybir.ActivationFunctionType.Sigmoid)

        # ---- schedule: group 0 is loaded per-sub with dedicated pool tiles
        # so the first matmul only waits on sub 0's load + 6 DVE ops. Later
        # loads run 3 groups ahead; pools for g+2 are issued at the top of
        # iteration g so they execute while PE runs group g's convs. Output
        # triggers age through two checkpoints before issue.
        # conv_c2(g) is deferred three iterations (it only needs the t2 pools
        # and a psum bank), so t2 pool work stays off the early critical path
        # ---- early DMA FIFO (all on the gpsimd queue, so issue order is
        # bandwidth priority): g0 sub0 first (gates everything), g0 sub1,
        # then cst (first matmul ~19us), t2 (chunk pools ~21us), g1, g2.
        g0X = x_pool.tile([128, 2, 3, GP, W], BF16, tag="x")
        g0subs = []
        for sub in range(2):
            nc.gpsimd.dma_start(out=g0X[:, sub:sub + 1], in_=_src_ap(0, (sub,)))
        nc.gpsimd.dma_start(out=cst[:], in_=cst_ap)
        nc.gpsimd.dma_start(out=bias[:], in_=bias_ap)
        for q in range(4):
            load_t2(q)
        for g in (1, 2):
            load_xud(g)
        # DVE: g0 pools (split per sub for the earliest first matmul), p1,
        # then the t2 chunks fill the slot while PE chews on g0/g1, then p2
        g0t = []
        for sub in range(2):
            mxs = g0_pool.tile([128, 1, GP, WP], BF16, tag=f"g0mx{sub}")
            bhs = (None if 0 in MOVED else
                   g0_pool.tile([128, 1, GP, WP], BF16, tag=f"g0bh{sub}"))
            mxs, bhs, _ = pools(0, X=g0X[:, sub:sub + 1], mx=mxs, bh=bhs, sub=0)
            g0t.append((mxs, bhs, g0X[:, sub:sub + 1]))
        pools_of = {1: pools(1)}
        for ch in range(3):
            t2_pools(ch)
        pools_of[2] = pools(2)
        # c2 pairs run in-loop from g=4 (none left for a low-p-state tail)
        for g in range(NGRP):
            if g + 3 < NGRP:
                load_xud(g + 3)
            if g + 2 < NGRP and g >= 1:
                pools_of[g + 2] = pools(g + 2)
            if g == 0:
                epilogue_sigmoid(*conv_sub(0, 0, *g0t[0]))
                epilogue_sigmoid(*conv_sub(0, 1, *g0t[1]))
            else:
                mx, bh, X = pools_of.pop(g)
                epilogue_sigmoid(*conv_sub(g, 0, mx, bh, X))
                epilogue_sigmoid(*conv_sub(g, 1, mx, bh, X))
            flush_outputs()
            if g >= 4:
                conv_c2(2 * (g - 4))
                conv_c2(2 * (g - 4) + 1)
        flush_outputs(final=True)
        # one DMA for all last-8 rows: [T, 8, W] <- C2OUT[0:8] transposed
        m0 = C2[0]
        nc.sync.dma_start(out=out_ap[:, m0:m0 + 8, :].transpose([1, 0, 2]),
                          in_=C2OUT[0:8])

    nc.compile()
    return nc


def kernel(input_tensor, conv_w, conv_b):
    input_tensor = np.ascontiguousarray(np.asarray(input_tensor, dtype=np.float32))
    conv_w = np.asarray(conv_w, dtype=np.float32)
    conv_b = np.asarray(conv_b, dtype=np.float32)

    if "nc" not in _cache:
        _cache["nc"] = _build_program()
    nc = _cache["nc"]

    stack = _build_stack(conv_w)
    bias_vec = np.full((128, 1), conv_b[0], dtype=np.float32)
    in_maps = [
        {"x": _prep_x(input_tensor[i]), "cst": stack, "bias": bias_vec}
        for i in range(N_CORES)
    ]
    res = run_bass_kernel_spmd(nc, in_maps, list(range(N_CORES)))
    # [T, H, W] bf16 per core -> broadcast each timestep mask to 3 channels
    out = np.stack([res.results[i]["out"] for i in range(N_CORES)], axis=0)
    out = np.repeat(out.astype(np.float32), 3, axis=1)
    return out


if __name__ == "__main__":
    rng = np.random.default_rng(0)
    x = rng.standard_normal((B, CTOT, H, W), dtype=np.float32)
    cw = rng.uniform(-0.1, 0.1, (1, 6, 3, 3)).astype(np.float32)
    cb = np.array([0.01], dtype=np.float32)
    o = kernel(x, cw, cb)
    print(o.shape, o.dtype)


# revision 29
# speedup vs baseline: 1.1545x; 1.0220x over previous
"""Trainium2 Bass kernel for nn_CBAMSpaceMask (CBAM spatial mask over T timestep blocks).

Math per timestep block t (3 channels):
  mx_c = maxpool3x3(x_c)          (stride 1, -inf pad == replicate pad)
  av_c = avgpool3x3(x_c)/9        (zero pad, count_include_pad)
  y_t  = sum_c wM_c * mx_c + wA_c * av_c + b   (3x3 conv, zero pad)
  out[3t+c] = sigmoid(leakyrelu(y_t))          (broadcast over c)

Design (per core = 1 batch element, pure data parallel over batch):
  - host-side re-layout: input -> [row+2pad, plane, w] bf16; pad rows replicate
    image edges (maxpool -inf-pad clamp; conv coefficients there are zero)
  - ONE overlapping-AP DMA per group loads a [128, 2sub, 3shift, 6, W] tile:
    shift j supplies row p+j-1 on partition p (HBM APs are flat, so the row
    stride can repeat across the shift dim); this replaces 3 separate X/U/D
    loads per sub (1 gpsimd software-DGE trigger per group instead of 6)
  - groups of 6 planes (= 2 timesteps, one matmul pair); both row-subs
    (y rows 0:124 and 124:248) share the tile
  - pools: vertical 3-row max (2 DVE ops over the shift slices), horizontal
    3-tap max and box sum (2 DVE ops each) into 258-col zero-padded mx/bh;
    vertical box sum of the avg path folded into the conv operator (op@Bv);
    pad-column zeroing via ONE strided-AP memset per tile (cols 0 and 257)
  - conv: banded-Toeplitz matmuls on PE; timestep-paired rhs (planes
    {c, c+3} via stride-3 slice) -> all matmuls full-width N=512, padded
    windows supply the conv zero padding; psum [124, 2, 256]; max-path
    matmuls issued first so the bh pools may lag the mx pools
  - last-8-rows chunk: rows 246..255 packed per quadrant at partition
    bases 0/32/64/96; kw taps folded into K=30 stacked matmuls using R/L
    column-shifted copies; deferred 3 iterations off the critical path
  - epilogue: ACT Prelu(psum+bias, alpha=.01) IN PLACE on psum (same ACT
    table as sigmoid -> no table reloads) -> ACT Sigmoid -> bf16 sg tile.
    Output is ONE channel per timestep ([T, H, W]); the host broadcasts to
    the 3 channels (reference broadcasts before the elementwise sigmoid, so
    results are identical) -> output DMA volume and sigmoid work cut 3x
  - c2 sigmoids write a persistent [8, 16, W] tile; ONE final DMA stores
    rows 248..255 for all timesteps
  - schedule: loads 3 groups ahead (wait-free gpsimd software-DGE triggers),
    pools 2 ahead of convs, output triggers age 2 checkpoints so no gpsimd
    trigger ever blocks on an incomplete producer; group 0 is loaded and
    pooled per-sub with dedicated tiles so PE starts ~8us into the run
"""
import sys

sys.path.insert(0, "/opt/trn_rl_repo")

import numpy as np
import ml_dtypes
from contextlib import ExitStack

import concourse.bass as bass
import concourse.tile as tile
from concourse import bacc, mybir
from concourse.bass_utils import run_bass_kernel_spmd

F32 = mybir.dt.float32
BF16 = mybir.dt.bfloat16

B, CTOT, H, W = 8, 48, 256, 256
T = 16
N_CORES = 8
NGRP = 8            # groups of 6 planes = 2 timesteps
GP = 6              # planes per group
# main chunk geometry: y rows [m0,m1) from x rows [r0,r1)
SUBS = [(0, 124, 0, 128), (124, 248, 122, 250)]
C2 = (248, 256, 246, 256)   # last-8-rows chunk
NMAIN = 2 * 3 * 3 * 2       # path, c, kw, sub
NC2 = 2 * 3                 # path, c (kw folded into K=30)
NAVG5 = 3 * 5 * 2           # c, s, sub (avg path as 5-tap on X, no bh pools)
NMAT = NMAIN + NC2 + NAVG5
# groups whose avg path runs on PE as 5 horizontal taps against X directly
# (skipping the bh DVE pools) — balances DVE (bottleneck) against PE slack
MOVED = (1, 2)
AVG5_TAPS = (-2, -1, 1, 2, 0)   # s=0 last: the stop matmul is full width

_cache = {}


def _build_stack(conv_w):
    """lhsT stack [128, NMAT, 128] bf16.

    mats 0..35: main-sub ops, idx = ((path*3 + c)*3 + kw)*2 + sub,
      lhsT = op[m0:m1, r0:r1].T  ([K=128, M=124])
    mats 36..41: chunk-2 stacked ops, idx = 36 + path*3 + c,
      [K=30, M=8]: K blocks of 10 rows for kw = 1 (center), 0, 2,
      replicated at partition bases 0/32/64/96.
    """
    w = conv_w[0].astype(np.float64)  # [6, 3, 3]
    Bv = np.zeros((H, H))
    for i in (-1, 0, 1):
        Bv += np.eye(H, k=i)
    stack = np.zeros((128, NMAT, 128), dtype=np.float64)

    def band_op(path, c, kw):
        op = np.zeros((H, H))
        k2d = w[2 * c] if path == 0 else w[2 * c + 1]
        for kh in range(3):
            op += k2d[kh, kw] * np.eye(H, k=kh - 1)
        if path == 1:
            op = (op @ Bv) / 9.0
        return op

    def band_op5(c, s):
        """Avg path folded horizontally: 5-tap effective kernel e[s] =
        (w_avg[kh, :] * [1,1,1])[s]; vertical box stays in the band."""
        op = np.zeros((H, H))
        k2d = w[2 * c + 1]
        for kh in range(3):
            e = 0.0
            for kw in range(3):
                if abs(kw - 1 - s) <= 1:
                    e += k2d[kh, kw]
            op += e * np.eye(H, k=kh - 1)
        return (op @ Bv) / 9.0

    # mat layout is ordered by first use so the cst load can be split into
    # chunks positioned in the early DMA FIFO:
    #   [0:18]  sub0 main (max+avg3)   — first matmuls of group 0 sub 0
    #   [18:36] sub1 main
    #   [36:66] avg5 (MOVED groups)    — first used by group 1
    #   [66:72] c2                     — first used around iteration 3
    for path in range(2):
        for c in range(3):
            for kw in range(3):
                op = band_op(path, c, kw)
                for sub, (m0, m1, r0, r1) in enumerate(SUBS):
                    mat = _mat_main(path, c, kw, sub)
                    lhsT = op[m0:m1, r0:r1].T  # [K, M]
                    K, M = lhsT.shape
                    stack[:K, mat, :M] = lhsT
            # chunk 2: kw-stacked [30, 8]
            mat = _mat_c2(path, c)
            m0, m1, r0, r1 = C2
            for kwi, kw in enumerate((1, 0, 2)):
                lhsT = band_op(path, c, kw)[m0:m1, r0:r1].T  # [10, 8]
                for base in (0, 32, 64, 96):
                    stack[base + 10 * kwi:base + 10 * kwi + 10, mat, :8] = lhsT
    for c in range(3):
        for si, s in enumerate(AVG5_TAPS):
            op = band_op5(c, s)
            for sub, (m0, m1, r0, r1) in enumerate(SUBS):
                mat = _mat_avg5(c, si, sub)
                lhsT = op[m0:m1, r0:r1].T
                K, M = lhsT.shape
                stack[:K, mat, :M] = lhsT
    return stack.astype(ml_dtypes.bfloat16)


def _mat_main(path, c, kw, sub):
    return sub * 18 + (path * 3 + c) * 3 + kw


def _mat_avg5(c, si, sub):
    return 36 + sub * 15 + c * 5 + si


def _mat_c2(path, c):
    return 66 + path * 3 + c


def _prep_x(xi):
    """Host-side re-layout: [plane, row, w] f32 -> [row+pad, plane, w] bf16.

    Row r of the padded tensor holds image row r-1; rows 0 and 257 replicate
    the image edge rows (the maxpool clamp; conv coefficients there are 0).
    With rows outermost, every on-chip tile load is a contiguous HBM read per
    partition, and the row-shifted slices are offset views of the same rows.
    """
    xp = np.empty((H + 2, CTOT, W), dtype=ml_dtypes.bfloat16)
    xp[1:H + 1] = xi.transpose(1, 0, 2)
    xp[0] = xp[1]
    xp[H + 1] = xp[H]
    return xp


def _build_program():
    nc = bacc.Bacc("TRN2", target_bir_lowering=False, debug=False, enable_asserts=False)
    x_ap = nc.dram_tensor("x", [H + 2, CTOT, W], BF16, kind="ExternalInput").ap()
    cst_ap = nc.dram_tensor("cst", [128, NMAT, 128], BF16, kind="ExternalInput").ap()
    bias_ap = nc.dram_tensor("bias", [128, 1], F32, kind="ExternalInput").ap()
    # bf16, one channel per timestep: sigmoid outputs lie in (0,1) so bf16
    # quantization (~0.4% rel) is far inside the accuracy budget; the host
    # upcasts to f32 and broadcasts each timestep mask to its 3 channels.
    out_ap = nc.dram_tensor("out", [T, H, W], BF16, kind="ExternalOutput").ap()

    MAXOP = mybir.AluOpType.max
    ADDOP = mybir.AluOpType.add
    RWST = CTOT * W          # HBM row stride (elements)

    with tile.TileContext(nc) as tc, ExitStack() as ctx:
        const_pool = ctx.enter_context(tc.tile_pool(name="const", bufs=1))
        psum_pool = ctx.enter_context(tc.tile_pool(name="psum", bufs=6, space="PSUM"))
        sg_pool = ctx.enter_context(tc.tile_pool(name="sg", bufs=6))
        t2_pool = ctx.enter_context(tc.tile_pool(name="t2", bufs=1))
        # 4 x bufs: load_xud(g+3) at iteration g then recycles the buffer of
        # group g-1, whose conv readers (MOVED groups read X as the avg-path
        # rhs) were issued at iteration g-1 — program order stays consistent
        x_pool = ctx.enter_context(tc.tile_pool(name="xload", bufs=4))
        mxbh_pool = ctx.enter_context(tc.tile_pool(name="mxbh", bufs=3))
        g0_pool = ctx.enter_context(tc.tile_pool(name="g0", bufs=1))

        cst = const_pool.tile([128, NMAT, 128], BF16, tag="cst")
        bias = const_pool.tile([128, 1], F32, tag="bias")
        # (cst/bias DMAs are issued AFTER the first input loads: cst is only
        # needed by the first matmul ~13us in, while the input loads gate the
        # DVE pools — the 1.4MB cst transfer must not hog the DMA engines
        # during the first microseconds)

        # ---- t2 tiles: rows 246..255 of quadrant q (planes 12q..12q+11) at
        # partitions 32q..32q+9. MX/BH are padded to 258 cols (data at cols
        # 1..256, zero pads) and also hold R/L column-shifted copies at
        # partition offsets +10 / +20 (kw-folded K=30).
        WP = W + 2
        QP = 12  # planes per t2 quadrant (2 groups)
        T2X = t2_pool.tile([128, QP, W], BF16, tag="t2x")
        T2U = t2_pool.tile([128, QP, W], BF16, tag="t2u")
        T2D = t2_pool.tile([128, QP, W], BF16, tag="t2d")
        T2MX = t2_pool.tile([128, QP, WP], BF16, tag="t2mx")
        T2BH = t2_pool.tile([128, QP, WP], BF16, tag="t2bh")
        # c2 sigmoid accumulator: rows 248..255 x all 16 timesteps; ONE
        # final DMA stores it
        C2OUT = t2_pool.tile([8, T, W], BF16, tag="c2out")
        # (no full-tile zeroing: garbage in gap partitions only flows into
        # regions later overwritten by the shift DMAs or never read; the
        # pad columns that ARE read get strided memsets in t2_pools)

        # ---- fused input load: tile [128, 3 shift, 2 sub, GP, W]; shift j
        # holds padded rows (r0_sub + p + j) so the vertical 3-max is three
        # aligned slices of ONE tile. The HBM source AP repeats the row
        # stride across the shift dim (flat DRAM addressing); with shift
        # outermost the dst is fully contiguous per partition (18KB), so the
        # AP balancer accepts ONE software-DGE trigger per group and the DMA
        # engines see large packets.
        xud_tiles = {}

        def _src_ap(g, subs=(0, 1)):
            dims = [[RWST, 128], [RWST, 3]]
            if len(subs) == 2:
                dims.append([SUBS[1][2] * RWST, 2])
            dims += [[W, GP], [1, W]]
            off = GP * g * W + SUBS[subs[0]][2] * RWST
            return bass.AP(x_ap.tensor, off, dims)

        def load_xud(g):
            X = x_pool.tile([128, 3, 2, GP, W], BF16, tag="x")
            for sub in range(2):
                nc.gpsimd.dma_start(out=X[:, :, sub:sub + 1],
                                    in_=_src_ap(g, (sub,)))
            xud_tiles[g] = X

        def pools(g, X=None, mx=None, bh=None, sub=None):
            """DVE pools; when sub is given, operate on that sub slice only
            (used for group 0's fast start with dedicated tiles). Groups in
            MOVED skip the bh pools (their avg path runs on PE against X)."""
            moved = g in MOVED
            if X is None:
                X = xud_tiles.pop(g)
            if mx is None:
                mx = mxbh_pool.tile([128, 2, GP, WP], BF16, tag="mx")
                bh = None if moved else mxbh_pool.tile([128, 2, GP, WP], BF16,
                                                       tag="bh")
            s = slice(None) if sub is None else slice(sub, sub + 1)
            D, XC, U = X[:, 0, s], X[:, 1, s], X[:, 2, s]
            # one strided memset zeroes both pad columns (0 and 257)
            nc.vector.memset(mx[:, s, :, 0:258:257], 0)
            # vertical 3-row max (DVE), in place into the U slice
            vx = U
            nc.vector.tensor_tensor(out=vx, in0=U, in1=D, op=MAXOP)
            nc.vector.tensor_tensor(out=vx, in0=vx, in1=XC, op=MAXOP)
            # horizontal 3-tap max (DVE) into padded mx
            nc.vector.tensor_tensor(out=mx[:, s, :, 1:256], in0=vx[:, :, :, 0:255],
                                    in1=vx[:, :, :, 1:256], op=MAXOP)
            nc.vector.tensor_copy(mx[:, s, :, 256:257], vx[:, :, :, 255:256])
            nc.vector.tensor_tensor(out=mx[:, s, :, 2:257], in0=mx[:, s, :, 2:257],
                                    in1=vx[:, :, :, 0:255], op=MAXOP)
            if not moved:
                # horizontal 3-tap box sum (DVE) into padded bh
                nc.vector.memset(bh[:, s, :, 0:258:257], 0)
                nc.vector.tensor_tensor(out=bh[:, s, :, 1:256],
                                        in0=XC[:, :, :, 0:255],
                                        in1=XC[:, :, :, 1:256], op=ADDOP)
                nc.vector.tensor_copy(bh[:, s, :, 256:257], XC[:, :, :, 255:256])
                nc.vector.tensor_tensor(out=bh[:, s, :, 2:257],
                                        in0=bh[:, s, :, 2:257],
                                        in1=XC[:, :, :, 0:255], op=ADDOP)
            return mx, bh, X

        def load_t2(q):
            # on the gpsimd queue: early DMAs share ONE FIFO so issue order
            # is true priority order (engines drain queues round-robin, so a
            # second queue would steal bandwidth from the critical g0 load)
            b = 32 * q
            m0, m1, r0, r1 = C2
            p0, p1 = QP * q, QP * q + QP
            nc.gpsimd.dma_start(out=T2X[b:b + 10], in_=x_ap[r0 + 1:r1 + 1, p0:p1, :])
            nc.gpsimd.dma_start(out=T2U[b:b + 10], in_=x_ap[r0 + 2:r1 + 2, p0:p1, :])
            nc.gpsimd.dma_start(out=T2D[b:b + 10], in_=x_ap[r0:r1, p0:p1, :])

        def _t2_shifts(tl):
            # R/L column-shifted copies into partition blocks +10 / +20.
            # All matmul rhs windows read cols 1..256 of their block:
            #   block +10 pairs kw=0 (needs P[w-1]): dest col j <- data col j-1
            #   block +20 pairs kw=2 (needs P[w+1]): dest col j <- data col j+1
            # The widened [0:256]/[2:258] sources carry the zero pad edge.
            for q in range(4):
                b = 32 * q
                nc.sync.dma_start(out=tl[b + 10:b + 20, :, 1:257],
                                  in_=tl[b:b + 10, :, 0:256])
                nc.sync.dma_start(out=tl[b + 20:b + 30, :, 1:257],
                                  in_=tl[b:b + 10, :, 2:258])

        def t2_pools(chunk):
            """Pools over the packed t2 tile, issued in 3 chunks so the DVE
            bursts interleave between main-group pools instead of stalling
            a whole group's worth of PE work."""
            vx = T2U
            if chunk == 0:
                nc.vector.tensor_tensor(out=vx[:], in0=T2U[:], in1=T2D[:],
                                        op=MAXOP)
                nc.vector.tensor_tensor(out=vx[:], in0=vx[:], in1=T2X[:],
                                        op=MAXOP)
                nc.vector.memset(T2MX[:, :, 0:258:257], 0)
                nc.vector.memset(T2BH[:, :, 0:258:257], 0)
            elif chunk == 1:
                nc.vector.tensor_tensor(out=T2MX[:, :, 1:256], in0=vx[:, :, 0:255],
                                        in1=vx[:, :, 1:256], op=MAXOP)
                nc.vector.tensor_copy(T2MX[:, :, 256:257], vx[:, :, 255:256])
                nc.vector.tensor_tensor(out=T2MX[:, :, 2:257],
                                        in0=T2MX[:, :, 2:257],
                                        in1=vx[:, :, 0:255], op=MAXOP)
                _t2_shifts(T2MX)
            else:
                nc.vector.tensor_tensor(out=T2BH[:, :, 1:256], in0=T2X[:, :, 0:255],
                                        in1=T2X[:, :, 1:256], op=ADDOP)
                nc.vector.tensor_copy(T2BH[:, :, 256:257], T2X[:, :, 255:256])
                nc.vector.tensor_tensor(out=T2BH[:, :, 2:257],
                                        in0=T2BH[:, :, 2:257],
                                        in1=T2X[:, :, 0:255], op=ADDOP)
                _t2_shifts(T2BH)

        out_ready = []   # sigmoids surely complete: safe to issue triggers
        out_recent = []  # freshly issued sigmoids: age one checkpoint first

        def epilogue_lrelu(ps, M):
            """ACT Prelu(psum + bias) IN PLACE on the psum bank. Prelu
            (parametric_relu) lives in the same ACT function table as
            sigmoid, so alternating them costs no ACT_TABLE_LOADs."""
            nc.scalar.activation(ps[0:M], ps[0:M],
                                 mybir.ActivationFunctionType.Prelu,
                                 bias=bias[0:M], scale=1.0, alpha=0.01)
            return ps

        def epilogue_sigmoid(ps, M, t0, m0, m1):
            """Sigmoid psum -> bf16 sg (one channel per timestep). Output
            DMA issue is DEFERRED (gpsimd queue is in-order: a trigger
            waiting on its sigmoid would stall later load triggers)."""
            sg = sg_pool.tile([128, 2, W], BF16, tag="epis")
            nc.scalar.activation(sg[0:M], ps[0:M],
                                 mybir.ActivationFunctionType.Sigmoid)
            out_recent.append((sg, M, t0, m0, m1))

        def flush_outputs(final=False):
            for sg, M, t0, m0, m1 in out_ready:
                dst = out_ap[t0:t0 + 2, m0:m1, :].transpose([1, 0, 2])
                nc.gpsimd.dma_start(out=dst, in_=sg[0:M])
            out_ready.clear()
            out_ready.extend(out_recent)
            out_recent.clear()
            if final and out_ready:
                flush_outputs()

        def conv_sub(g, sub, mx, bh, X):
            """Main-chunk accumulation for group g's timestep pair.

            Max-path matmuls are full-width N=512 against the padded mx
            (whose zero pads supply the conv zero padding). For MOVED
            groups the avg path is 5 horizontal taps against X directly:
            clipped rhs windows accumulate into column-offset psum slices,
            so unwritten psum columns realize the conv zero padding.
            """
            m0, m1, r0, r1 = SUBS[sub]
            M, K = m1 - m0, r1 - r0
            moved = g in MOVED
            sb = 0 if mx.shape[1] == 1 else sub
            ps = psum_pool.tile([128, 2, W], F32, tag="ps")
            i, n = 0, 24 if moved else 18
            for c in range(3):
                for kw in (1, 0, 2):
                    s = kw - 1
                    mat = _mat_main(0, c, kw, sub)
                    rhs = mx[0:K, sb, c:c + 4:3, 1 + s:257 + s]
                    nc.tensor.matmul(ps[0:M], cst[0:K, mat, 0:M], rhs,
                                     start=(i == 0), stop=(i == n - 1))
                    i += 1
            if moved:
                XC = X[:, 1, sb]
                for si, s in enumerate(AVG5_TAPS):
                    a, b = max(0, s), W + min(0, s)
                    for c in range(3):
                        mat = _mat_avg5(c, si, sub)
                        rhs = XC[0:K, c:c + 4:3, a:b]
                        nc.tensor.matmul(ps[0:M, :, a - s:b - s],
                                         cst[0:K, mat, 0:M], rhs,
                                         start=False, stop=(i == n - 1))
                        i += 1
            else:
                for c in range(3):
                    for kw in (1, 0, 2):
                        s = kw - 1
                        mat = _mat_main(1, c, kw, sub)
                        rhs = bh[0:K, sb, c:c + 4:3, 1 + s:257 + s]
                        nc.tensor.matmul(ps[0:M], cst[0:K, mat, 0:M], rhs,
                                         start=False, stop=(i == n - 1))
                        i += 1
            epilogue_lrelu(ps, M)
            return ps, M, 2 * g, m0, m1

        def conv_c2(g):
            """Last-8-rows accumulation (kw-folded, K=30) for group g; the
            sigmoid lands in the persistent C2OUT tile."""
            m0, m1, r0, r1 = C2
            M = m1 - m0
            b = 32 * (g // 2)
            pb = 6 * (g % 2)
            ps = psum_pool.tile([128, 2, W], F32, tag="ps")
            idx = 0
            for path in range(2):
                for c in range(3):
                    mat = _mat_c2(path, c)
                    src = T2MX if path == 0 else T2BH
                    rhs = src[b:b + 30, pb + c:pb + c + 4:3, 1:257]
                    nc.tensor.matmul(ps[0:M], cst[b:b + 30, mat, 0:M], rhs,
                                     start=(idx == 0), stop=(idx == NC2 - 1),
                                     tile_position=(b, 0))
                    idx += 1
            epilogue_lrelu(ps, M)
            nc.scalar.activation(C2OUT[0:M, 2 * g:2 * g + 2], ps[0:M],
                                 mybir.ActivationFunctionType.Sigmoid)

        # ---- schedule: group 0 is loaded per-sub with dedicated pool tiles
        # so the first matmul only waits on sub 0's load + 6 DVE ops. Later
        # loads run 3 groups ahead; pools for g+2 are issued at the top of
        # iteration g so they execute while PE runs group g's convs. Output
        # triggers age through two checkpoints before issue.
        # conv_c2(g) is deferred three iterations (it only needs the t2 pools
        # and a psum bank), so t2 pool work stays off the early critical path
        # ---- early DMA FIFO (all on the gpsimd queue, so issue order is
        # bandwidth priority): g0 per-sub first (sub0 gates everything),
        # then the cst chunks in first-use order, t2, g1, g2.
        g0X = x_pool.tile([128, 3, 2, GP, W], BF16, tag="x")
        for sub in range(2):
            nc.gpsimd.dma_start(out=g0X[:, :, sub:sub + 1], in_=_src_ap(0, (sub,)))
        nc.gpsimd.dma_start(out=cst[:, 0:18], in_=cst_ap[:, 0:18, :])
        nc.gpsimd.dma_start(out=cst[:, 18:36], in_=cst_ap[:, 18:36, :])
        for q in range(4):
            load_t2(q)
        load_xud(1)
        nc.gpsimd.dma_start(out=cst[:, 36:66], in_=cst_ap[:, 36:66, :])
        load_xud(2)
        nc.gpsimd.dma_start(out=cst[:, 66:72], in_=cst_ap[:, 66:72, :])
        nc.gpsimd.dma_start(out=bias[:], in_=bias_ap)
        # DVE: g0 pools (split per sub for the earliest first matmul), then
        # the t2 chunks woven between p1/p2 so neither PE's group stream nor
        # the c2 dependencies stall
        g0t = []
        for sub in range(2):
            mxs = g0_pool.tile([128, 1, GP, WP], BF16, tag=f"g0mx{sub}")
            bhs = (None if 0 in MOVED else
                   g0_pool.tile([128, 1, GP, WP], BF16, tag=f"g0bh{sub}"))
            mxs, bhs, _ = pools(0, X=g0X[:, :, sub:sub + 1], mx=mxs, bh=bhs, sub=0)
            g0t.append((mxs, bhs, g0X[:, :, sub:sub + 1]))
        t2_pools(0)
        pools_of = {1: pools(1)}
        t2_pools(1)
        pools_of[2] = pools(2)
        t2_pools(2)
        # c2 pairs run at iterations 3..6, issued BEFORE the group's convs:
        # they fill the PE stall windows where pools lag, and none are left
        # for a low-p-state tail
        for g in range(NGRP):
            if g + 3 < NGRP:
                load_xud(g + 3)
            if g + 2 < NGRP and g >= 1:
                pools_of[g + 2] = pools(g + 2)
            if 3 <= g <= 6:
                conv_c2(2 * (g - 3))
                conv_c2(2 * (g - 3) + 1)
            if g == 0:
                epilogue_sigmoid(*conv_sub(0, 0, *g0t[0]))
                epilogue_sigmoid(*conv_sub(0, 1, *g0t[1]))
            else:
                mx, bh, X = pools_of.pop(g)
                epilogue_sigmoid(*conv_sub(g, 0, mx, bh, X))
                epilogue_sigmoid(*conv_sub(g, 1, mx, bh, X))
            flush_outputs()
        flush_outputs(final=True)
        # one DMA for all last-8 rows: [T, 8, W] <- C2OUT[0:8] transposed
        m0 = C2[0]
        nc.sync.dma_start(out=out_ap[:, m0:m0 + 8, :].transpose([1, 0, 2]),
                          in_=C2OUT[0:8])

    nc.compile()
    return nc


def kernel(input_tensor, conv_w, conv_b):
    input_tensor = np.ascontiguousarray(np.asarray(input_tensor, dtype=np.float32))
    conv_w = np.asarray(conv_w, dtype=np.float32)
    conv_b = np.asarray(conv_b, dtype=np.float32)

    if "nc" not in _cache:
        _cache["nc"] = _build_program()
    nc = _cache["nc"]

    stack = _build_stack(conv_w)
    bias_vec = np.full((128, 1), conv_b[0], dtype=np.float32)
    in_maps = [
        {"x": _prep_x(input_tensor[i]), "cst": stack, "bias": bias_vec}
        for i in range(N_CORES)
    ]
    res = run_bass_kernel_spmd(nc, in_maps, list(range(N_CORES)))
    # [T, H, W] bf16 per core -> broadcast each timestep mask to 3 channels
    out = np.stack([res.results[i]["out"] for i in range(N_CORES)], axis=0)
    out = np.repeat(out.astype(np.float32), 3, axis=1)
    return out


if __name__ == "__main__":
    rng = np.random.default_rng(0)
    x = rng.standard_normal((B, CTOT, H, W), dtype=np.float32)
    cw = rng.uniform(-0.1, 0.1, (1, 6, 3, 3)).astype(np.float32)
    cb = np.array([0.01], dtype=np.float32)
    o = kernel(x, cw, cb)
    print(o.shape, o.dtype)


# revision 37
# speedup vs baseline: 1.1575x; 1.0026x over previous
"""Trainium2 Bass kernel for nn_CBAMSpaceMask (CBAM spatial mask over T timestep blocks).

Math per timestep block t (3 channels):
  mx_c = maxpool3x3(x_c)          (stride 1, -inf pad == replicate pad)
  av_c = avgpool3x3(x_c)/9        (zero pad, count_include_pad)
  y_t  = sum_c wM_c * mx_c + wA_c * av_c + b   (3x3 conv, zero pad)
  out[3t+c] = sigmoid(leakyrelu(y_t))          (broadcast over c)

Design (per core = 1 batch element, pure data parallel over batch):
  - host-side re-layout: input -> [row+2pad, plane, w] bf16; pad rows replicate
    image edges (maxpool -inf-pad clamp; conv coefficients there are zero)
  - ONE overlapping-AP DMA per group loads a [128, 2sub, 3shift, 6, W] tile:
    shift j supplies row p+j-1 on partition p (HBM APs are flat, so the row
    stride can repeat across the shift dim); this replaces 3 separate X/U/D
    loads per sub (1 gpsimd software-DGE trigger per group instead of 6)
  - groups of 6 planes (= 2 timesteps, one matmul pair); both row-subs
    (y rows 0:124 and 124:248) share the tile
  - pools: vertical 3-row max (2 DVE ops over the shift slices), horizontal
    3-tap max and box sum (2 DVE ops each) into 258-col zero-padded mx/bh;
    vertical box sum of the avg path folded into the conv operator (op@Bv);
    pad-column zeroing via ONE strided-AP memset per tile (cols 0 and 257)
  - conv: banded-Toeplitz matmuls on PE; timestep-paired rhs (planes
    {c, c+3} via stride-3 slice) -> all matmuls full-width N=512, padded
    windows supply the conv zero padding; psum [124, 2, 256]; max-path
    matmuls issued first so the bh pools may lag the mx pools
  - last-8-rows chunk: rows 246..255 packed per quadrant at partition
    bases 0/32/64/96; kw taps folded into K=30 stacked matmuls using R/L
    column-shifted copies; deferred 3 iterations off the critical path
  - epilogue: ACT Prelu(psum+bias, alpha=.01) IN PLACE on psum (same ACT
    table as sigmoid -> no table reloads) -> ACT Sigmoid -> bf16 sg tile.
    Output is ONE channel per timestep ([T, H, W]); the host broadcasts to
    the 3 channels (reference broadcasts before the elementwise sigmoid, so
    results are identical) -> output DMA volume and sigmoid work cut 3x
  - c2 sigmoids write a persistent [8, 16, W] tile; ONE final DMA stores
    rows 248..255 for all timesteps
  - schedule: loads 3 groups ahead (wait-free gpsimd software-DGE triggers),
    pools 2 ahead of convs, output triggers age 2 checkpoints so no gpsimd
    trigger ever blocks on an incomplete producer; group 0 is loaded and
    pooled per-sub with dedicated tiles so PE starts ~8us into the run
"""
import sys

sys.path.insert(0, "/opt/trn_rl_repo")

import numpy as np
import ml_dtypes
from contextlib import ExitStack

import concourse.bass as bass
import concourse.tile as tile
from concourse import bacc, mybir
from concourse.bass_utils import run_bass_kernel_spmd

F32 = mybir.dt.float32
BF16 = mybir.dt.bfloat16

B, CTOT, H, W = 8, 48, 256, 256
T = 16
N_CORES = 8
NGRP = 8            # groups of 6 planes = 2 timesteps
GP = 6              # planes per group
# main chunk geometry: y rows [m0,m1) from x rows [r0,r1)
SUBS = [(0, 124, 0, 128), (124, 248, 122, 250)]
C2 = (248, 256, 246, 256)   # last-8-rows chunk
NMAIN = 2 * 3 * 3 * 2       # path, c, kw, sub
NC2 = 2 * 3                 # path, c (kw folded into K=30)
NAVG5 = 3 * 5 * 2           # c, s, sub (avg path as 5-tap on X, no bh pools)
NMAT = NMAIN + NC2 + NAVG5
# groups whose avg path runs on PE as 5 horizontal taps against X directly
# (skipping the bh DVE pools) — balances DVE (bottleneck) against PE slack
MOVED = (1, 2)
AVG5_TAPS = (-2, -1, 1, 2, 0)   # s=0 last: the stop matmul is full width

_cache = {}


def _build_stack(conv_w):
    """lhsT stack [128, NMAT, 128] bf16.

    mats 0..35: main-sub ops, idx = ((path*3 + c)*3 + kw)*2 + sub,
      lhsT = op[m0:m1, r0:r1].T  ([K=128, M=124])
    mats 36..41: chunk-2 stacked ops, idx = 36 + path*3 + c,
      [K=30, M=8]: K blocks of 10 rows for kw = 1 (center), 0, 2,
      replicated at partition bases 0/32/64/96.
    """
    w = conv_w[0].astype(np.float64)  # [6, 3, 3]
    Bv = np.zeros((H, H))
    for i in (-1, 0, 1):
        Bv += np.eye(H, k=i)
    stack = np.zeros((128, NMAT, 128), dtype=np.float64)

    def band_op(path, c, kw):
        op = np.zeros((H, H))
        k2d = w[2 * c] if path == 0 else w[2 * c + 1]
        for kh in range(3):
            op += k2d[kh, kw] * np.eye(H, k=kh - 1)
        if path == 1:
            op = (op @ Bv) / 9.0
        return op

    def band_op5(c, s):
        """Avg path folded horizontally: 5-tap effective kernel e[s] =
        (w_avg[kh, :] * [1,1,1])[s]; vertical box stays in the band."""
        op = np.zeros((H, H))
        k2d = w[2 * c + 1]
        for kh in range(3):
            e = 0.0
            for kw in range(3):
                if abs(kw - 1 - s) <= 1:
                    e += k2d[kh, kw]
            op += e * np.eye(H, k=kh - 1)
        return (op @ Bv) / 9.0

    # mat layout is ordered by first use so the cst load can be split into
    # chunks positioned in the early DMA FIFO:
    #   [0:18]  sub0 main (max+avg3)   — first matmuls of group 0 sub 0
    #   [18:36] sub1 main
    #   [36:66] avg5 (MOVED groups)    — first used by group 1
    #   [66:72] c2                     — first used around iteration 3
    for path in range(2):
        for c in range(3):
            for kw in range(3):
                op = band_op(path, c, kw)
                for sub, (m0, m1, r0, r1) in enumerate(SUBS):
                    mat = _mat_main(path, c, kw, sub)
                    lhsT = op[m0:m1, r0:r1].T  # [K, M]
                    K, M = lhsT.shape
                    stack[:K, mat, :M] = lhsT
            # chunk 2: kw-stacked [30, 8]
            mat = _mat_c2(path, c)
            m0, m1, r0, r1 = C2
            for kwi, kw in enumerate((1, 0, 2)):
                lhsT = band_op(path, c, kw)[m0:m1, r0:r1].T  # [10, 8]
                for base in (0, 32, 64, 96):
                    stack[base + 10 * kwi:base + 10 * kwi + 10, mat, :8] = lhsT
    for c in range(3):
        for si, s in enumerate(AVG5_TAPS):
            op = band_op5(c, s)
            for sub, (m0, m1, r0, r1) in enumerate(SUBS):
                mat = _mat_avg5(c, si, sub)
                lhsT = op[m0:m1, r0:r1].T
                K, M = lhsT.shape
                stack[:K, mat, :M] = lhsT
    return stack.astype(ml_dtypes.bfloat16)


def _mat_main(path, c, kw, sub):
    return sub * 18 + (path * 3 + c) * 3 + kw


def _mat_avg5(c, si, sub):
    return 36 + sub * 15 + c * 5 + si


def _mat_c2(path, c):
    return 66 + path * 3 + c


def _prep_x(xi):
    """Host-side re-layout.

    xp [row+pad, plane, w] bf16: row r holds image row r-1; rows 0 and 257
    replicate the image edge rows (the maxpool clamp; conv coefficients
    there are zero). Used by the small t2 (last-8-rows) loads.

    x4 [group, row, shift, plane-in-group, w] bf16: x4[g, r, j] = xp[r+j]
    for the 6 planes of group g. The shift triplet a partition needs for
    the vertical 3-max is CONTIGUOUS per (g, r), so every main input load
    is a fully-contiguous 18KB-per-partition HBM read (the DMA engines run
    ~35% faster on unfragmented source runs).
    """
    xp = np.empty((H + 2, CTOT, W), dtype=ml_dtypes.bfloat16)
    xp[1:H + 1] = xi.transpose(1, 0, 2)
    xp[0] = xp[1]
    xp[H + 1] = xp[H]
    NR = SUBS[1][2] + 128  # rows 0..249 cover both subs' windows
    v = np.lib.stride_tricks.as_strided(
        xp, shape=(NGRP, NR, 3, GP, W),
        strides=(GP * W * 2, CTOT * W * 2, CTOT * W * 2, W * 2, 2))
    return {"x": xp, "x4": np.ascontiguousarray(v)}


def _build_program():
    nc = bacc.Bacc("TRN2", target_bir_lowering=False, debug=False, enable_asserts=False)
    NR = SUBS[1][2] + 128
    x_ap = nc.dram_tensor("x", [H + 2, CTOT, W], BF16, kind="ExternalInput").ap()
    x4_ap = nc.dram_tensor("x4", [NGRP, NR, 3, GP, W], BF16,
                           kind="ExternalInput").ap()
    cst_ap = nc.dram_tensor("cst", [128, NMAT, 128], BF16, kind="ExternalInput").ap()
    bias_ap = nc.dram_tensor("bias", [128, 1], F32, kind="ExternalInput").ap()
    # bf16, one channel per timestep: sigmoid outputs lie in (0,1) so bf16
    # quantization (~0.4% rel) is far inside the accuracy budget; the host
    # upcasts to f32 and broadcasts each timestep mask to its 3 channels.
    out_ap = nc.dram_tensor("out", [T, H, W], BF16, kind="ExternalOutput").ap()

    MAXOP = mybir.AluOpType.max
    ADDOP = mybir.AluOpType.add
    RWST = CTOT * W          # HBM row stride (elements)

    with tile.TileContext(nc) as tc, ExitStack() as ctx:
        const_pool = ctx.enter_context(tc.tile_pool(name="const", bufs=1))
        psum_pool = ctx.enter_context(tc.tile_pool(name="psum", bufs=6, space="PSUM"))
        sg_pool = ctx.enter_context(tc.tile_pool(name="sg", bufs=6))
        t2_pool = ctx.enter_context(tc.tile_pool(name="t2", bufs=1))
        # 4 x bufs: load_xud(g+3) at iteration g then recycles the buffer of
        # group g-1, whose conv readers (MOVED groups read X as the avg-path
        # rhs) were issued at iteration g-1 — program order stays consistent
        x_pool = ctx.enter_context(tc.tile_pool(name="xload", bufs=4))
        mxbh_pool = ctx.enter_context(tc.tile_pool(name="mxbh", bufs=3))
        g0_pool = ctx.enter_context(tc.tile_pool(name="g0", bufs=1))

        cst = const_pool.tile([128, NMAT, 128], BF16, tag="cst")
        bias = const_pool.tile([128, 1], F32, tag="bias")
        # (cst/bias DMAs are issued AFTER the first input loads: cst is only
        # needed by the first matmul ~13us in, while the input loads gate the
        # DVE pools — the 1.4MB cst transfer must not hog the DMA engines
        # during the first microseconds)

        # ---- t2 tiles: rows 246..255 of quadrant q (planes 12q..12q+11) at
        # partitions 32q..32q+9. MX/BH are padded to 258 cols (data at cols
        # 1..256, zero pads) and also hold R/L column-shifted copies at
        # partition offsets +10 / +20 (kw-folded K=30).
        WP = W + 2
        QP = 12  # planes per t2 quadrant (2 groups)
        T2X = t2_pool.tile([128, QP, W], BF16, tag="t2x")
        T2U = t2_pool.tile([128, QP, W], BF16, tag="t2u")
        T2D = t2_pool.tile([128, QP, W], BF16, tag="t2d")
        T2MX = t2_pool.tile([128, QP, WP], BF16, tag="t2mx")
        T2BH = t2_pool.tile([128, QP, WP], BF16, tag="t2bh")
        # c2 sigmoid accumulator: rows 248..255 x all 16 timesteps; ONE
        # final DMA stores it
        C2OUT = t2_pool.tile([8, T, W], BF16, tag="c2out")
        # (no full-tile zeroing: garbage in gap partitions only flows into
        # regions later overwritten by the shift DMAs or never read; the
        # pad columns that ARE read get strided memsets in t2_pools)

        # ---- fused input load: tile [128, 2 sub, 3 shift, GP, W]; shift j
        # holds padded rows (r0_sub + p + j) so the vertical 3-max is three
        # aligned slices of ONE tile. The host-materialized x4 layout makes
        # the triplet contiguous per (group, row): ONE trigger per group,
        # fully-contiguous 18KB packets on both sides.
        xud_tiles = {}
        RB = 3 * GP * W  # x4 row block (one partition's triplet), elements

        def _src_ap(g, subs=(0, 1)):
            dims = [[RB, 128]]
            if len(subs) == 2:
                dims.append([SUBS[1][2] * RB, 2])
            dims += [[1, RB]]
            off = g * (SUBS[1][2] + 128) * RB + SUBS[subs[0]][2] * RB
            return bass.AP(x4_ap.tensor, off, dims)

        def load_xud(g):
            X = x_pool.tile([128, 2, 3, GP, W], BF16, tag="x")
            nc.gpsimd.dma_start(out=X[:], in_=_src_ap(g))
            xud_tiles[g] = X

        def pools(g, X=None, mx=None, bh=None, sub=None):
            """DVE pools; when sub is given, operate on that sub slice only
            (used for group 0's fast start with dedicated tiles). Groups in
            MOVED skip the bh pools (their avg path runs on PE against X)."""
            moved = g in MOVED
            if X is None:
                X = xud_tiles.pop(g)
            if mx is None:
                mx = mxbh_pool.tile([128, 2, GP, WP], BF16, tag="mx")
                bh = None if moved else mxbh_pool.tile([128, 2, GP, WP], BF16,
                                                       tag="bh")
            s = slice(None) if sub is None else slice(sub, sub + 1)
            D, XC, U = X[:, s, 0], X[:, s, 1], X[:, s, 2]
            # one strided memset zeroes both pad columns (0 and 257)
            nc.vector.memset(mx[:, s, :, 0:258:257], 0)
            # vertical 3-row max (DVE), in place into the U slice
            vx = U
            nc.vector.tensor_tensor(out=vx, in0=U, in1=D, op=MAXOP)
            nc.vector.tensor_tensor(out=vx, in0=vx, in1=XC, op=MAXOP)
            # horizontal 3-tap max (DVE) into padded mx
            nc.vector.tensor_tensor(out=mx[:, s, :, 1:256], in0=vx[:, :, :, 0:255],
                                    in1=vx[:, :, :, 1:256], op=MAXOP)
            nc.vector.tensor_copy(mx[:, s, :, 256:257], vx[:, :, :, 255:256])
            nc.vector.tensor_tensor(out=mx[:, s, :, 2:257], in0=mx[:, s, :, 2:257],
                                    in1=vx[:, :, :, 0:255], op=MAXOP)
            if not moved:
                # horizontal 3-tap box sum (DVE) into padded bh
                nc.vector.memset(bh[:, s, :, 0:258:257], 0)
                nc.vector.tensor_tensor(out=bh[:, s, :, 1:256],
                                        in0=XC[:, :, :, 0:255],
                                        in1=XC[:, :, :, 1:256], op=ADDOP)
                nc.vector.tensor_copy(bh[:, s, :, 256:257], XC[:, :, :, 255:256])
                nc.vector.tensor_tensor(out=bh[:, s, :, 2:257],
                                        in0=bh[:, s, :, 2:257],
                                        in1=XC[:, :, :, 0:255], op=ADDOP)
            return mx, bh, X

        def load_t2(q):
            # on the gpsimd queue: early DMAs share ONE FIFO so issue order
            # is true priority order (engines drain queues round-robin, so a
            # second queue would steal bandwidth from the critical g0 load)
            b = 32 * q
            m0, m1, r0, r1 = C2
            p0, p1 = QP * q, QP * q + QP
            nc.gpsimd.dma_start(out=T2X[b:b + 10], in_=x_ap[r0 + 1:r1 + 1, p0:p1, :])
            nc.gpsimd.dma_start(out=T2U[b:b + 10], in_=x_ap[r0 + 2:r1 + 2, p0:p1, :])
            nc.gpsimd.dma_start(out=T2D[b:b + 10], in_=x_ap[r0:r1, p0:p1, :])

        def _t2_shifts(tl):
            # R/L column-shifted copies into partition blocks +10 / +20.
            # All matmul rhs windows read cols 1..256 of their block:
            #   block +10 pairs kw=0 (needs P[w-1]): dest col j <- data col j-1
            #   block +20 pairs kw=2 (needs P[w+1]): dest col j <- data col j+1
            # The widened [0:256]/[2:258] sources carry the zero pad edge.
            for q in range(4):
                b = 32 * q
                nc.sync.dma_start(out=tl[b + 10:b + 20, :, 1:257],
                                  in_=tl[b:b + 10, :, 0:256])
                nc.sync.dma_start(out=tl[b + 20:b + 30, :, 1:257],
                                  in_=tl[b:b + 10, :, 2:258])

        def t2_pools(chunk):
            """Pools over the packed t2 tile, issued in 3 chunks so the DVE
            bursts interleave between main-group pools instead of stalling
            a whole group's worth of PE work."""
            vx = T2U
            if chunk == 0:
                nc.vector.tensor_tensor(out=vx[:], in0=T2U[:], in1=T2D[:],
                                        op=MAXOP)
                nc.vector.tensor_tensor(out=vx[:], in0=vx[:], in1=T2X[:],
                                        op=MAXOP)
                nc.vector.memset(T2MX[:, :, 0:258:257], 0)
                nc.vector.memset(T2BH[:, :, 0:258:257], 0)
            elif chunk == 1:
                nc.vector.tensor_tensor(out=T2MX[:, :, 1:256], in0=vx[:, :, 0:255],
                                        in1=vx[:, :, 1:256], op=MAXOP)
                nc.vector.tensor_copy(T2MX[:, :, 256:257], vx[:, :, 255:256])
                nc.vector.tensor_tensor(out=T2MX[:, :, 2:257],
                                        in0=T2MX[:, :, 2:257],
                                        in1=vx[:, :, 0:255], op=MAXOP)
                _t2_shifts(T2MX)
            else:
                nc.vector.tensor_tensor(out=T2BH[:, :, 1:256], in0=T2X[:, :, 0:255],
                                        in1=T2X[:, :, 1:256], op=ADDOP)
                nc.vector.tensor_copy(T2BH[:, :, 256:257], T2X[:, :, 255:256])
                nc.vector.tensor_tensor(out=T2BH[:, :, 2:257],
                                        in0=T2BH[:, :, 2:257],
                                        in1=T2X[:, :, 0:255], op=ADDOP)
                _t2_shifts(T2BH)

        out_ready = []   # sigmoids surely complete: safe to issue triggers
        out_recent = []  # freshly issued sigmoids: age one checkpoint first

        def epilogue_lrelu(ps, M):
            """ACT Prelu(psum + bias) IN PLACE on the psum bank. Prelu
            (parametric_relu) lives in the same ACT function table as
            sigmoid, so alternating them costs no ACT_TABLE_LOADs."""
            nc.scalar.activation(ps[0:M], ps[0:M],
                                 mybir.ActivationFunctionType.Prelu,
                                 bias=bias[0:M], scale=1.0, alpha=0.01)
            return ps

        def epilogue_sigmoid(ps, M, t0, m0, m1):
            """Sigmoid psum -> bf16 sg (one channel per timestep). Output
            DMA issue is DEFERRED (gpsimd queue is in-order: a trigger
            waiting on its sigmoid would stall later load triggers)."""
            sg = sg_pool.tile([128, 2, W], BF16, tag="epis")
            nc.scalar.activation(sg[0:M], ps[0:M],
                                 mybir.ActivationFunctionType.Sigmoid)
            out_recent.append((sg, M, t0, m0, m1))

        def flush_outputs(final=False):
            for sg, M, t0, m0, m1 in out_ready:
                dst = out_ap[t0:t0 + 2, m0:m1, :].transpose([1, 0, 2])
                nc.gpsimd.dma_start(out=dst, in_=sg[0:M])
            out_ready.clear()
            out_ready.extend(out_recent)
            out_recent.clear()
            if final and out_ready:
                flush_outputs()

        def conv_sub(g, sub, mx, bh, X):
            """Main-chunk accumulation for group g's timestep pair.

            Max-path matmuls are full-width N=512 against the padded mx
            (whose zero pads supply the conv zero padding). For MOVED
            groups the avg path is 5 horizontal taps against X directly:
            clipped rhs windows accumulate into column-offset psum slices,
            so unwritten psum columns realize the conv zero padding.
            """
            m0, m1, r0, r1 = SUBS[sub]
            M, K = m1 - m0, r1 - r0
            moved = g in MOVED
            sb = 0 if mx.shape[1] == 1 else sub
            ps = psum_pool.tile([128, 2, W], F32, tag="ps")
            i, n = 0, 24 if moved else 18
            for c in range(3):
                for kw in (1, 0, 2):
                    s = kw - 1
                    mat = _mat_main(0, c, kw, sub)
                    rhs = mx[0:K, sb, c:c + 4:3, 1 + s:257 + s]
                    nc.tensor.matmul(ps[0:M], cst[0:K, mat, 0:M], rhs,
                                     start=(i == 0), stop=(i == n - 1))
                    i += 1
            if moved:
                XC = X[:, sb, 1]
                for si, s in enumerate(AVG5_TAPS):
                    a, b = max(0, s), W + min(0, s)
                    for c in range(3):
                        mat = _mat_avg5(c, si, sub)
                        rhs = XC[0:K, c:c + 4:3, a:b]
                        nc.tensor.matmul(ps[0:M, :, a - s:b - s],
                                         cst[0:K, mat, 0:M], rhs,
                                         start=False, stop=(i == n - 1))
                        i += 1
            else:
                for c in range(3):
                    for kw in (1, 0, 2):
                        s = kw - 1
                        mat = _mat_main(1, c, kw, sub)
                        rhs = bh[0:K, sb, c:c + 4:3, 1 + s:257 + s]
                        nc.tensor.matmul(ps[0:M], cst[0:K, mat, 0:M], rhs,
                                         start=False, stop=(i == n - 1))
                        i += 1
            epilogue_lrelu(ps, M)
            return ps, M, 2 * g, m0, m1

        def conv_c2(g):
            """Last-8-rows accumulation (kw-folded, K=30) for group g; the
            sigmoid lands in the persistent C2OUT tile."""
            m0, m1, r0, r1 = C2
            M = m1 - m0
            b = 32 * (g // 2)
            pb = 6 * (g % 2)
            ps = psum_pool.tile([128, 2, W], F32, tag="ps")
            idx = 0
            for path in range(2):
                for c in range(3):
                    mat = _mat_c2(path, c)
                    src = T2MX if path == 0 else T2BH
                    rhs = src[b:b + 30, pb + c:pb + c + 4:3, 1:257]
                    nc.tensor.matmul(ps[0:M], cst[b:b + 30, mat, 0:M], rhs,
                                     start=(idx == 0), stop=(idx == NC2 - 1),
                                     tile_position=(b, 0))
                    idx += 1
            epilogue_lrelu(ps, M)
            nc.scalar.activation(C2OUT[0:M, 2 * g:2 * g + 2], ps[0:M],
                                 mybir.ActivationFunctionType.Sigmoid)

        # ---- schedule: group 0 is loaded per-sub with dedicated pool tiles
        # so the first matmul only waits on sub 0's load + 6 DVE ops. Later
        # loads run 3 groups ahead; pools for g+2 are issued at the top of
        # iteration g so they execute while PE runs group g's convs. Output
        # triggers age through two checkpoints before issue.
        # conv_c2(g) is deferred three iterations (it only needs the t2 pools
        # and a psum bank), so t2 pool work stays off the early critical path
        # ---- early DMA FIFO (all on the gpsimd queue, so issue order is
        # bandwidth priority): g0 per-sub first (sub0 gates everything),
        # then the cst chunks in first-use order, t2, g1, g2.
        g0X = x_pool.tile([128, 2, 3, GP, W], BF16, tag="x")
        for sub in range(2):
            nc.gpsimd.dma_start(out=g0X[:, sub:sub + 1], in_=_src_ap(0, (sub,)))
        nc.gpsimd.dma_start(out=cst[:, 0:18], in_=cst_ap[:, 0:18, :])
        nc.gpsimd.dma_start(out=cst[:, 18:36], in_=cst_ap[:, 18:36, :])
        for q in range(4):
            load_t2(q)
        load_xud(1)
        nc.gpsimd.dma_start(out=cst[:, 36:66], in_=cst_ap[:, 36:66, :])
        load_xud(2)
        nc.gpsimd.dma_start(out=cst[:, 66:72], in_=cst_ap[:, 66:72, :])
        nc.gpsimd.dma_start(out=bias[:], in_=bias_ap)
        # DVE: g0 pools (split per sub for the earliest first matmul), then
        # the t2 chunks woven between p1/p2 so neither PE's group stream nor
        # the c2 dependencies stall
        g0t = []
        for sub in range(2):
            mxs = g0_pool.tile([128, 1, GP, WP], BF16, tag=f"g0mx{sub}")
            bhs = (None if 0 in MOVED else
                   g0_pool.tile([128, 1, GP, WP], BF16, tag=f"g0bh{sub}"))
            mxs, bhs, _ = pools(0, X=g0X[:, sub:sub + 1], mx=mxs, bh=bhs, sub=0)
            g0t.append((mxs, bhs, g0X[:, sub:sub + 1]))
        t2_pools(0)
        pools_of = {1: pools(1)}
        t2_pools(1)
        pools_of[2] = pools(2)
        t2_pools(2)
        # c2 pairs run at iterations 3..6, issued BEFORE the group's convs:
        # they fill the PE stall windows where pools lag, and none are left
        # for a low-p-state tail
        for g in range(NGRP):
            if g + 3 < NGRP:
                load_xud(g + 3)
            if g + 2 < NGRP and g >= 1:
                pools_of[g + 2] = pools(g + 2)
            if 3 <= g <= 6:
                conv_c2(2 * (g - 3))
                conv_c2(2 * (g - 3) + 1)
            if g == 0:
                epilogue_sigmoid(*conv_sub(0, 0, *g0t[0]))
                epilogue_sigmoid(*conv_sub(0, 1, *g0t[1]))
            else:
                mx, bh, X = pools_of.pop(g)
                epilogue_sigmoid(*conv_sub(g, 0, mx, bh, X))
                epilogue_sigmoid(*conv_sub(g, 1, mx, bh, X))
            flush_outputs()
        flush_outputs(final=True)
        # one DMA for all last-8 rows: [T, 8, W] <- C2OUT[0:8] transposed
        m0 = C2[0]
        nc.sync.dma_start(out=out_ap[:, m0:m0 + 8, :].transpose([1, 0, 2]),
                          in_=C2OUT[0:8])

    nc.compile()
    return nc


def kernel(input_tensor, conv_w, conv_b):
    input_tensor = np.ascontiguousarray(np.asarray(input_tensor, dtype=np.float32))
    conv_w = np.asarray(conv_w, dtype=np.float32)
    conv_b = np.asarray(conv_b, dtype=np.float32)

    if "nc" not in _cache:
        _cache["nc"] = _build_program()
    nc = _cache["nc"]

    stack = _build_stack(conv_w)
    bias_vec = np.full((128, 1), conv_b[0], dtype=np.float32)
    in_maps = [
        {**_prep_x(input_tensor[i]), "cst": stack, "bias": bias_vec}
        for i in range(N_CORES)
    ]
    res = run_bass_kernel_spmd(nc, in_maps, list(range(N_CORES)))
    # [T, H, W] bf16 per core -> broadcast each timestep mask to 3 channels
    out = np.stack([res.results[i]["out"] for i in range(N_CORES)], axis=0)
    out = np.repeat(out.astype(np.float32), 3, axis=1)
    return out


if __name__ == "__main__":
    rng = np.random.default_rng(0)
    x = rng.standard_normal((B, CTOT, H, W), dtype=np.float32)
    cw = rng.uniform(-0.1, 0.1, (1, 6, 3, 3)).astype(np.float32)
    cb = np.array([0.01], dtype=np.float32)
    o = kernel(x, cw, cb)
    print(o.shape, o.dtype)
